# revision 17
# baseline (speedup 1.0000x reference)
"""2-layer GAT (GATConv x2 + log_softmax) on 8 Trainium2 NeuronCores.

fp16 streaming design (SPMD across 8 cores; host does index marshaling
between launches, device does all arithmetic):
  - Host bin-packs the 20000 nodes into 160 windows of 128 dst slots each
    (20 windows/core), balancing in-degree so every window holds ~2125
    edges -> K = ceil(max/128) = 17 chunks of 128 edges.
  - Launch A: h = x@W1 with the attention alphas fused in via host-extended
    weights [W1 | W1@asd_src | W1@asd_dst]; emits a per-node fp16 table.
  - Host pre-gathers per-edge rows (halo exchange): for each edge slot a
    656-col fp16 record [h_lo(256) | h_hi(256) | asrc(8) | adst(8) |
    one-hot dst selector(128)].
  - Launch B: per window one bulk DMA; ACT computes exp(leakyrelu(alpha)),
    DVE scales messages by the per-head coefficients (sliced per k-group so
    selector matmuls start early); PE scatter-adds messages+denominators
    into PSUM via the shipped selectors; flush: divide, ELU, @[W2 | W2@a2]
    -> layer-2 fp16 table rows.
  - Launch C: layer 2 (H=1) folds the coefficient into the selector
    (CMPX = onehot(dl) * ex), so the 256-wide messages stream raw into the
    matmul; flush = log_softmax.
All matmul operands fp16 (1 cyc/row, f32 PSUM accumulation). All big
elementwise work stays on DVE: GpSimd shares SBUF ports with it, so
running both concurrently slows DVE ~2.7x (measured) and nets nothing.
"""
import heapq
import numpy as np
from contextlib import ExitStack

import concourse.bass as bass
import concourse.tile as tile
from concourse import mybir
from concourse.bass_utils import run_bass_kernel_spmd

F16 = mybir.dt.float16
F32 = mybir.dt.float32
I32 = mybir.dt.int32
AF = mybir.ActivationFunctionType
OP = mybir.AluOpType
P = 128
NCORES = 8
NWC = 20                 # windows per core
NEG_SLOPE = 0.2
RS1 = 656                # layer-1 record: 256+256+8+8 + one-hot(128)
MS1 = 520                # layer-1 message block: msg(512) + ex(8)
RS2 = 260                # layer-2 record: 256 + 1.0 + asrc + adst + dl
SPIN = 24                # PE warm-up matmuls (HAM releases the clock gate
                         # only after ~3.4us of sustained PE activity)


def _split_excess_waits(nc, max_waits=1):
    """This walrus build rejects instructions with >~2 sync waits; move excess
    waits onto same-engine wait-only instructions placed just before."""
    cnt = 0
    for f in nc.m.functions:
        for bb in f.blocks:
            new_insts = []
            for inst in bb.instructions:
                si = inst.sync_info
                if si is not None and si.on_wait and len(si.on_wait) > max_waits:
                    waits = list(si.on_wait)
                    extra, keep = waits[:-max_waits], waits[-max_waits:]
                    for w in extra:
                        cnt += 1
                        nop = mybir.InstNoOp(name=f"wsplit-{cnt}-{inst.name}", ins=[], outs=[])
                        nop.engine = inst.engine
                        nop.sync_info = mybir.SyncInfo(on_wait=[w], on_update=[])
                        new_insts.append(nop)
                    si.on_wait = keep
                new_insts.append(inst)
            bb.instructions = new_insts
    return cnt


def _pack_windows(dst, N, nw):
    """Greedy balance in-degree over nw windows of P slots. Returns
    win_of[N], slot_of[N], K (edge chunks per window)."""
    deg = np.bincount(dst, minlength=N)
    order = np.argsort(-deg, kind="stable")
    wload = np.zeros(nw, np.int64)
    wcnt = np.zeros(nw, np.int64)
    win_of = np.zeros(N, np.int32)
    slot_of = np.zeros(N, np.int32)
    heap = [(0, w) for w in range(nw)]
    heapq.heapify(heap)
    for n in order:
        while True:
            load, w = heapq.heappop(heap)
            if wcnt[w] < P:
                break
        win_of[n] = w
        slot_of[n] = wcnt[w]
        wcnt[w] += 1
        wload[w] += deg[n]
        if wcnt[w] < P:
            heapq.heappush(heap, (wload[w], w))
    K = int(np.ceil(wload.max() / P))
    return win_of, slot_of, K


def _kgroups(K, ng):
    """Split range(K) into ng contiguous groups for DVE/PE pipelining."""
    out = []
    base = 0
    for g in range(ng):
        n = (K - base + (ng - g) - 1) // (ng - g)
        out.append((base, base + n))
        base += n
    return out


def _spin_init(nc, const):
    a = const.tile([P, P], F16, tag="spin_a")
    nc.vector.memset(a[:], 1.0)
    b = const.tile([P, 512], F16, tag="spin_b")
    nc.vector.memset(b[:], 0.5)
    return a, b


def _spin(nc, spin_ab, pool, n):
    """Dependency-free matmuls that keep the PE HAM activity monitor busy
    (the clock gate drops to 1.2 GHz after ~3.4us of low PE activity);
    issued ahead of real matmuls so they fill operand-wait gaps."""
    a, b = spin_ab
    ps = pool.tile([P, 512], F32, tag="spin_ps")
    for i in range(n):
        nc.tensor.matmul(out=ps[:], lhsT=a[:], rhs=b[:], start=i == 0,
                         stop=i == n - 1)


def _build_a(D1, NPC):
    """h = x@W1 + fused alphas. out row = [po0(264) | po1(264)] =
    [h[0:256], asrc(8), h[256:512], adst(8)]."""
    nc = bass.Bass("TRN2", target_bir_lowering=False, debug=False, num_devices=NCORES)
    xT = nc.dram_tensor("xT", [D1, NPC], F16, kind="ExternalInput")
    W1E = nc.dram_tensor("W1E", [D1, 2 * 264], F16, kind="ExternalInput")
    tab1 = nc.dram_tensor("tab1", [NPC, 528], F16, kind="ExternalOutput")
    KB = D1 // P
    with tile.TileContext(nc) as tc:
        with ExitStack() as ctx:
            const = ctx.enter_context(tc.tile_pool(name="const", bufs=1))
            work = ctx.enter_context(tc.tile_pool(name="work", bufs=3))
            ps0 = ctx.enter_context(tc.tile_pool(name="ps0", bufs=2, space="PSUM"))
            ps1 = ctx.enter_context(tc.tile_pool(name="ps1", bufs=2, space="PSUM"))
            psW = ctx.enter_context(tc.tile_pool(name="psW", bufs=1, space="PSUM"))
            spin_ab = _spin_init(nc, const)
            _spin(nc, spin_ab, psW, SPIN)
            xsb, w0sb, w1sb = [], [], []
            for kb in range(KB):
                t = const.tile([P, NPC], F16, tag=f"x_{kb}")
                nc.sync.dma_start(out=t[:], in_=xT[kb * P:(kb + 1) * P, :])
                xsb.append(t)
                t0 = const.tile([P, 264], F16, tag=f"w0_{kb}")
                nc.sync.dma_start(out=t0[:], in_=W1E[kb * P:(kb + 1) * P, 0:264])
                w0sb.append(t0)
                t1 = const.tile([P, 264], F16, tag=f"w1_{kb}")
                nc.sync.dma_start(out=t1[:], in_=W1E[kb * P:(kb + 1) * P, 264:528])
                w1sb.append(t1)
            for t_i in range(NPC // P):
                po0 = ps0.tile([P, 264], F32, tag="po0")
                po1 = ps1.tile([P, 264], F32, tag="po1")
                for kb in range(KB):
                    lhsT = xsb[kb][:, t_i * P:(t_i + 1) * P]
                    nc.tensor.matmul(out=po0[:], lhsT=lhsT, rhs=w0sb[kb][:],
                                     start=kb == 0, stop=kb == KB - 1)
                for kb in range(KB):
                    lhsT = xsb[kb][:, t_i * P:(t_i + 1) * P]
                    nc.tensor.matmul(out=po1[:], lhsT=lhsT, rhs=w1sb[kb][:],
                                     start=kb == 0, stop=kb == KB - 1)
                stage = work.tile([P, 528], F16, tag="stage")
                nc.scalar.activation(out=stage[:, 0:264], in_=po0[:], func=AF.Copy)
                nc.vector.tensor_copy(out=stage[:, 264:528], in_=po1[:])
                nc.sync.dma_start(out=tab1[t_i * P:(t_i + 1) * P, :], in_=stage[:])
    _split_excess_waits(nc)
    return nc


def _build_b(K, D1, H1, OUTC, add_bias):
    """Layer-1 edge phase + flush into the layer-2 table.

    Elementwise work split: DVE keeps msg_lo scaling + part of the selector
    compares + the (PSUM-bound) flush; GpSimd takes msg_hi scaling, the rest
    of the compares, and the small copies. Both sliced into 4 k-groups so
    selector matmuls start early and the PE never idles long enough for the
    HAM clock gate to drop."""
    C1 = D1 // H1
    nc = bass.Bass("TRN2", target_bir_lowering=False, debug=False, num_devices=NCORES)
    EDG = nc.dram_tensor("EDG", [NWC, P, K * RS1], F16, kind="ExternalInput")
    W2E = nc.dram_tensor("W2E", [D1, OUTC + 2], F16, kind="ExternalInput")
    BB = nc.dram_tensor("BB", [P, D1], F32, kind="ExternalInput")
    tab2 = nc.dram_tensor("tab2", [NWC * P, OUTC + 2], F16, kind="ExternalOutput")
    KG = _kgroups(K, 4)
    with tile.TileContext(nc) as tc:
        with ExitStack() as ctx:
            const = ctx.enter_context(tc.tile_pool(name="const", bufs=1))
            gp = ctx.enter_context(tc.tile_pool(name="gp", bufs=3))
            mp = ctx.enter_context(tc.tile_pool(name="mp", bufs=2))
            cp = ctx.enter_context(tc.tile_pool(name="cp", bufs=2))
            sp = ctx.enter_context(tc.tile_pool(name="sp", bufs=2))
            fp = ctx.enter_context(tc.tile_pool(name="fp", bufs=2))
            st = ctx.enter_context(tc.tile_pool(name="st", bufs=2))
            ps0 = ctx.enter_context(tc.tile_pool(name="ps0", bufs=2, space="PSUM"))
            ps1 = ctx.enter_context(tc.tile_pool(name="ps1", bufs=2, space="PSUM"))
            psH = ctx.enter_context(tc.tile_pool(name="psH", bufs=2, space="PSUM"))
            psT = ctx.enter_context(tc.tile_pool(name="psT", bufs=1, space="PSUM"))
            psW = ctx.enter_context(tc.tile_pool(name="psW", bufs=1, space="PSUM"))
            spin_ab = _spin_init(nc, const)
            _spin(nc, spin_ab, psW, SPIN)

            iota_i = const.tile([P, P], I32)
            nc.gpsimd.iota(iota_i[:], pattern=[[1, P]], base=0, channel_multiplier=0)
            piota_i = const.tile([P, 1], I32)
            nc.gpsimd.iota(piota_i[:], pattern=[[0, 1]], base=0, channel_multiplier=1)
            piota_f = const.tile([P, 1], F32)
            nc.vector.tensor_copy(out=piota_f[:], in_=piota_i[:])
            iota_f = const.tile([P, P], F32)
            nc.vector.tensor_copy(out=iota_f[:], in_=iota_i[:])
            identF = const.tile([P, P], F32)
            nc.vector.tensor_tensor(out=identF[:], in0=iota_f[:],
                                    in1=piota_f[:].to_broadcast([P, P]), op=OP.is_equal)
            if add_bias:
                bb = const.tile([P, D1], F32)
                nc.sync.dma_start(out=bb[:], in_=BB[:, :])
            w2e_sb = []
            for cb in range(D1 // P):
                t = const.tile([P, OUTC + 2], F16, tag=f"w2e_{cb}")
                nc.sync.dma_start(out=t[:], in_=W2E[cb * P:(cb + 1) * P, :])
                w2e_sb.append(t)

            def flush_b(w, po0, po1):
                dr = fp.tile([P, H1], F32, tag="dr")
                nc.scalar.activation(out=dr[:], in_=po1[:, 256:264], func=AF.Copy,
                                     bias=1e-16)
                drr = fp.tile([P, H1], F32, tag="drr")
                nc.vector.reciprocal(out=drr[:], in_=dr[:])
                o1 = fp.tile([P, D1], F32, tag="o1")
                nc.vector.tensor_tensor(
                    out=o1[:, 0:256].rearrange("p (h c) -> p h c", h=4),
                    in0=po0[:, 0:256].rearrange("p (h c) -> p h c", h=4),
                    in1=drr[:, 0:4].to_broadcast([P, 4, C1]), op=OP.mult)
                nc.vector.tensor_tensor(
                    out=o1[:, 256:512].rearrange("p (h c) -> p h c", h=4),
                    in0=po1[:, 0:256].rearrange("p (h c) -> p h c", h=4),
                    in1=drr[:, 4:8].to_broadcast([P, 4, C1]), op=OP.mult)
                if add_bias:
                    nc.vector.tensor_tensor(out=o1[:], in0=o1[:], in1=bb[:], op=OP.add)
                ee = fp.tile([P, D1], F32, tag="ee")
                nc.scalar.activation(out=ee[:], in_=o1[:], func=AF.Exp)
                nc.vector.tensor_scalar(out=ee[:], in0=ee[:], scalar1=1.0,
                                        scalar2=-1.0, op0=OP.min, op1=OP.add)
                h2 = fp.tile([P, D1], F32, tag="h2")
                nc.vector.tensor_tensor(out=h2[:], in0=o1[:], in1=ee[:], op=OP.max)
                ph2 = psH.tile([P, OUTC + 2], F32, tag="ph2")
                for cb in range(D1 // P):
                    pt = psT.tile([P, P], F32, tag="pt")
                    nc.tensor.transpose(out=pt[:], in_=h2[:, cb * P:(cb + 1) * P],
                                        identity=identF[:])
                    h2t = cp.tile([P, P], F16, tag="h2t")
                    nc.scalar.activation(out=h2t[:], in_=pt[:], func=AF.Copy)
                    nc.tensor.matmul(out=ph2[:], lhsT=h2t[:], rhs=w2e_sb[cb][:],
                                     start=cb == 0, stop=cb == D1 // P - 1)
                stage = st.tile([P, OUTC + 2], F16, tag="stage")
                nc.scalar.activation(out=stage[:], in_=ph2[:], func=AF.Copy)
                nc.sync.dma_start(out=tab2[w * P:(w + 1) * P, :], in_=stage[:])

            pending = []
            for w in range(NWC):
                G = gp.tile([P, K * RS1], F16, tag="G")
                nc.sync.dma_start(out=G[:], in_=EDG[w])
                Gv = G[:].rearrange("p (k t) -> p k t", t=RS1)
                S = sp.tile([P, K * H1], F32, tag="S")
                nc.vector.tensor_tensor(
                    out=S[:].rearrange("p (k h) -> p k h", h=H1),
                    in0=Gv[:, :, 512:520], in1=Gv[:, :, 520:528], op=OP.add)
                LR = sp.tile([P, K * H1], F32, tag="LR")
                nc.scalar.activation(out=LR[:], in_=S[:], func=AF.Prelu, alpha=NEG_SLOPE)
                EX = sp.tile([P, K * H1], F16, tag="EX")
                nc.scalar.activation(out=EX[:], in_=LR[:], func=AF.Exp)
                EXv = EX[:].rearrange("p (k h) -> p k h", h=H1)

                po0 = ps0.tile([P, 256], F32, tag="po0")
                po1 = ps1.tile([P, 264], F32, tag="po1")
                for g, (k0, k1) in enumerate(KG):
                    L = k1 - k0
                    Mg = mp.tile([P, L * MS1], F16, tag=f"M{g}")
                    Mgv = Mg[:].rearrange("p (k t) -> p k t", t=MS1)
                    nc.scalar.activation(out=Mgv[:, :, 512:520],
                                         in_=EXv[:, k0:k1, :], func=AF.Copy)
                    nc.vector.tensor_tensor(
                        out=Mgv[:, :, 0:512].rearrange("p k (h c) -> p k h c", h=8),
                        in0=Gv[:, k0:k1, 0:512].rearrange("p k (h c) -> p k h c", h=8),
                        in1=EXv[:, k0:k1, :].to_broadcast([P, L, H1, C1]), op=OP.mult)
                    for k in range(L):
                        gk = k0 + k
                        lhsT = G[:, gk * RS1 + 528:gk * RS1 + 656]
                        nc.tensor.matmul(out=po0[:], lhsT=lhsT,
                                         rhs=Mg[:, k * MS1:k * MS1 + 256],
                                         start=gk == 0, stop=gk == K - 1)
                        nc.tensor.matmul(out=po1[:], lhsT=lhsT,
                                         rhs=Mg[:, k * MS1 + 256:(k + 1) * MS1],
                                         start=gk == 0, stop=gk == K - 1)

                pending.append((w, po0, po1))
                if len(pending) > 1:
                    flush_b(*pending.pop(0))
            flush_b(*pending.pop(0))
    _split_excess_waits(nc)
    return nc


def _build_c(K, OUTC, add_bias):
    """Layer-2 edge phase: coefficient folded into the selector
    (CMPX = onehot * ex), raw message rows stream straight into the
    matmul; flush = divide, (+b2,) log_softmax."""
    nc = bass.Bass("TRN2", target_bir_lowering=False, debug=False, num_devices=NCORES)
    EDG = nc.dram_tensor("EDG", [NWC, P, K * RS2], F16, kind="ExternalInput")
    BB = nc.dram_tensor("BB", [P, OUTC], F32, kind="ExternalInput")
    out_t = nc.dram_tensor("out", [NWC * P, OUTC], F32, kind="ExternalOutput")
    KG = _kgroups(K, 4)
    with tile.TileContext(nc) as tc:
        with ExitStack() as ctx:
            const = ctx.enter_context(tc.tile_pool(name="const", bufs=1))
            gp = ctx.enter_context(tc.tile_pool(name="gp", bufs=3))
            cp = ctx.enter_context(tc.tile_pool(name="cp", bufs=2))
            sp = ctx.enter_context(tc.tile_pool(name="sp", bufs=2))
            fp = ctx.enter_context(tc.tile_pool(name="fp", bufs=2))
            ps0 = ctx.enter_context(tc.tile_pool(name="ps0", bufs=2, space="PSUM"))
            psW = ctx.enter_context(tc.tile_pool(name="psW", bufs=1, space="PSUM"))
            spin_ab = _spin_init(nc, const)
            _spin(nc, spin_ab, psW, SPIN)

            iota_i = const.tile([P, P], I32)
            nc.gpsimd.iota(iota_i[:], pattern=[[1, P]], base=0, channel_multiplier=0)
            iotag = const.tile([P, K * P], F16)
            for k in range(K):
                nc.gpsimd.tensor_copy(out=iotag[:, k * P:(k + 1) * P], in_=iota_i[:])
            if add_bias:
                bb = const.tile([P, OUTC], F32)
                nc.sync.dma_start(out=bb[:], in_=BB[:, :])

            def flush_c(w, po):
                dr = fp.tile([P, 1], F32, tag="dr")
                nc.scalar.activation(out=dr[:], in_=po[:, 256:257], func=AF.Copy,
                                     bias=1e-16)
                drr = fp.tile([P, 1], F32, tag="drr")
                nc.vector.reciprocal(out=drr[:], in_=dr[:])
                z = fp.tile([P, OUTC], F32, tag="z")
                nc.vector.tensor_scalar(out=z[:], in0=po[:, 0:256], scalar1=drr[:, :1],
                                        scalar2=None, op0=OP.mult)
                if add_bias:
                    nc.vector.tensor_tensor(out=z[:], in0=z[:], in1=bb[:], op=OP.add)
                ee = fp.tile([P, OUTC], F32, tag="ee")
                se = fp.tile([P, 1], F32, tag="se")
                nc.scalar.activation(out=ee[:], in_=z[:], func=AF.Exp, accum_out=se[:])
                lse = fp.tile([P, 1], F32, tag="lse")
                nc.scalar.activation(out=lse[:], in_=se[:], func=AF.Ln)
                nc.vector.tensor_scalar(out=z[:], in0=z[:], scalar1=lse[:, :1],
                                        scalar2=None, op0=OP.subtract)
                nc.sync.dma_start(out=out_t[w * P:(w + 1) * P, :], in_=z[:])

            pending = []
            for w in range(NWC):
                G = gp.tile([P, K * RS2], F16, tag="G")
                nc.sync.dma_start(out=G[:], in_=EDG[w])
                Gv = G[:].rearrange("p (k t) -> p k t", t=RS2)
                S = sp.tile([P, K], F32, tag="S")
                nc.vector.tensor_tensor(
                    out=S[:].rearrange("p (k o) -> p k o", o=1),
                    in0=Gv[:, :, 257:258], in1=Gv[:, :, 258:259], op=OP.add)
                LR = sp.tile([P, K], F32, tag="LR")
                nc.scalar.activation(out=LR[:], in_=S[:], func=AF.Prelu, alpha=NEG_SLOPE)
                EX = sp.tile([P, K], F16, tag="EX")
                nc.scalar.activation(out=EX[:], in_=LR[:], func=AF.Exp)
                EXv = EX[:].rearrange("p (k o) -> p k o", o=1)

                po = ps0.tile([P, 257], F32, tag="po")
                cmp2 = {}
                for g2, (j0, j1) in enumerate(_kgroups(K, 2)):
                    L2 = j1 - j0
                    CMPg = cp.tile([P, L2 * P], F16, tag=f"C{g2}")
                    nc.vector.tensor_tensor(
                        out=CMPg[:].rearrange("p (k q) -> p k q", q=P),
                        in0=iotag[:, j0 * P:j1 * P].rearrange("p (k q) -> p k q", q=P),
                        in1=Gv[:, j0:j1, 259:260].rearrange("p k o -> p (k o)")
                            .to_broadcast([P, L2, P]),
                        op=OP.is_equal)
                    cmp2[g2] = (CMPg, j0)
                for g, (k0, k1) in enumerate(KG):
                    L = k1 - k0
                    _spin(nc, spin_ab, psW, 2)
                    CMPg, j0 = cmp2[g // 2]
                    CMXg = cp.tile([P, L * P], F16, tag=f"X{g}")
                    nc.vector.tensor_tensor(
                        out=CMXg[:].rearrange("p (k q) -> p k q", q=P),
                        in0=CMPg[:, (k0 - j0) * P:(k1 - j0) * P]
                            .rearrange("p (k q) -> p k q", q=P),
                        in1=EXv[:, k0:k1, :].to_broadcast([P, L, P]), op=OP.mult)
                    for k in range(L):
                        gk = k0 + k
                        nc.tensor.matmul(out=po[:], lhsT=CMXg[:, k * P:(k + 1) * P],
                                         rhs=G[:, gk * RS2:gk * RS2 + 257],
                                         start=gk == 0, stop=gk == K - 1)

                pending.append((w, po))
                if len(pending) > 1:
                    flush_c(*pending.pop(0))
            flush_c(*pending.pop(0))
    _split_excess_waits(nc)
    return nc


def kernel(x, edge_index, W1, att_src1, att_dst1, b1, W2, att_src2, att_dst2, b2):
    x = np.asarray(x, np.float32)
    edge_index = np.asarray(edge_index)
    W1d = np.asarray(W1, np.float64)
    W2d = np.asarray(W2, np.float64)
    as1 = np.asarray(att_src1, np.float64)
    ad1 = np.asarray(att_dst1, np.float64)
    as2 = np.asarray(att_src2, np.float64)
    ad2 = np.asarray(att_dst2, np.float64)
    b1 = np.asarray(b1, np.float32)
    b2 = np.asarray(b2, np.float32)
    N, D1 = x.shape
    H1, C1 = att_src1.shape
    OUTC = W2.shape[1]
    NW = NCORES * NWC
    NPC = NWC * P
    core_ids = list(range(NCORES))
    npc_in = N // NCORES

    src = np.concatenate([edge_index[0], np.arange(N)]).astype(np.int64)
    dst = np.concatenate([edge_index[1], np.arange(N)]).astype(np.int64)
    win_of, slot_of, K = _pack_windows(dst, N, NW)

    # edge -> (window, chunk, partition) in window-major stable order
    w_e = win_of[dst]
    eorder = np.argsort(w_e, kind="stable")
    sw = w_e[eorder]
    counts = np.bincount(sw, minlength=NW)
    starts = np.concatenate([[0], np.cumsum(counts)[:-1]])
    pos = np.arange(len(sw)) - starts[sw]
    k_e = (pos // P).astype(np.int64)
    p_e = (pos % P).astype(np.int64)
    s_e = src[eorder]
    d_e = dst[eorder]
    row_of_node = win_of.astype(np.int64) * P + slot_of  # global table row

    # ---- Launch A: per-node table [h_lo, asrc, h_hi, adst] ----
    asd_s = np.zeros((D1, H1))
    asd_d = np.zeros((D1, H1))
    for h in range(H1):
        asd_s[h * C1:(h + 1) * C1, h] = as1[h]
        asd_d[h * C1:(h + 1) * C1, h] = ad1[h]
    W1E = np.concatenate([W1d[:, 0:256], W1d @ asd_s, W1d[:, 256:512], W1d @ asd_d],
                         axis=1).astype(np.float16)
    nc_a = _build_a(D1, NPC)
    in_maps = []
    for c in range(NCORES):
        xo = np.zeros((NPC, D1), np.float16)
        xo[:npc_in] = x[c * npc_in:(c + 1) * npc_in].astype(np.float16)
        in_maps.append({"xT": np.ascontiguousarray(xo.T), "W1E": W1E})
    res_a = run_bass_kernel_spmd(nc_a, in_maps, core_ids)
    tab1 = np.concatenate([res_a.results[c]["tab1"][:npc_in] for c in range(NCORES)], axis=0)
    h_lo = tab1[:, 0:256]
    a_src_n = tab1[:, 256:264]
    h_hi = tab1[:, 264:520]
    a_dst_n = tab1[:, 520:528]

    # ---- Launch B: layer-1 edge phase ----
    W2E = np.concatenate([W2d, W2d @ as2.T, W2d @ ad2.T], axis=1).astype(np.float16)
    BB1 = np.tile(b1.reshape(1, D1), (P, 1))
    nc_b = _build_b(K, D1, H1, OUTC, bool(np.any(b1)))
    in_maps = []
    for c in range(NCORES):
        m = (sw >= c * NWC) & (sw < (c + 1) * NWC)
        lw, kk, pp = sw[m] - c * NWC, k_e[m], p_e[m]
        sm, dm = s_e[m], d_e[m]
        EDG = np.zeros((NWC, P, K, RS1), np.float16)
        EDG[lw, pp, kk, 0:256] = h_lo[sm]
        EDG[lw, pp, kk, 256:512] = h_hi[sm]
        EDG[lw, pp, kk, 512:520] = a_src_n[sm]
        EDG[lw, pp, kk, 520:528] = a_dst_n[dm]
        EDG[lw, pp, kk, 528 + slot_of[dm]] = 1.0
        in_maps.append({"EDG": EDG.reshape(NWC, P, K * RS1), "W2E": W2E, "BB": BB1})
    res_b = run_bass_kernel_spmd(nc_b, in_maps, core_ids)
    tab2 = np.concatenate([res_b.results[c]["tab2"] for c in range(NCORES)], axis=0)
    h2p = tab2[:, 0:256]
    a_src2_n = tab2[:, 256]
    a_dst2_n = tab2[:, 257]

    # ---- Launch C: layer-2 edge phase + log_softmax ----
    BB2 = np.tile(b2.reshape(1, OUTC), (P, 1))
    nc_c = _build_c(K, OUTC, bool(np.any(b2)))
    in_maps = []
    sr = row_of_node[s_e]
    dr_ = row_of_node[d_e]
    for c in range(NCORES):
        m = (sw >= c * NWC) & (sw < (c + 1) * NWC)
        lw, kk, pp = sw[m] - c * NWC, k_e[m], p_e[m]
        srm, drm = sr[m], dr_[m]
        EDG = np.zeros((NWC, P, K, RS2), np.float16)
        EDG[:, :, :, 259] = 255.0
        EDG[lw, pp, kk, 0:256] = h2p[srm]
        EDG[lw, pp, kk, 256] = 1.0
        EDG[lw, pp, kk, 257] = a_src2_n[srm]
        EDG[lw, pp, kk, 258] = a_dst2_n[drm]
        EDG[lw, pp, kk, 259] = slot_of[d_e[m]].astype(np.float16)
        in_maps.append({"EDG": EDG.reshape(NWC, P, K * RS2), "BB": BB2})
    res_c = run_bass_kernel_spmd(nc_c, in_maps, core_ids)
    rows = np.concatenate([res_c.results[c]["out"] for c in range(NCORES)], axis=0)
    return np.ascontiguousarray(rows[row_of_node]).astype(np.float32)


# revision 18
# speedup vs baseline: 1.0376x; 1.0376x over previous
"""2-layer GAT (GATConv x2 + log_softmax) on 8 Trainium2 NeuronCores.

fp16 streaming design (SPMD across 8 cores; host does index marshaling
between launches, device does all arithmetic):
  - Host bin-packs the 20000 nodes into 160 windows of 128 dst slots each
    (20 windows/core), balancing in-degree so every window holds ~2125
    edges -> K = ceil(max/128) = 17 chunks of 128 edges.
  - Launch A: h = x@W1 with the attention alphas fused in via host-extended
    weights [W1 | W1@asd_src | W1@asd_dst]; emits a per-node fp16 table.
  - Host pre-gathers per-edge rows (halo exchange): for each edge slot a
    656-col fp16 record [h_lo(256) | h_hi(256) | asrc(8) | adst(8) |
    one-hot dst selector(128)].
  - Launch B: per window one bulk DMA; ACT computes exp(leakyrelu(alpha)),
    DVE scales messages by the per-head coefficients (sliced per k-group so
    selector matmuls start early); PE scatter-adds messages+denominators
    into PSUM via the shipped selectors; flush: divide, ELU, @[W2 | W2@a2]
    -> layer-2 fp16 table rows.
  - Launch C: layer 2 (H=1) folds the coefficient into the selector
    (CMPX = onehot(dl) * ex), so the 256-wide messages stream raw into the
    matmul; flush = log_softmax.
All matmul operands fp16 (1 cyc/row, f32 PSUM accumulation). All big
elementwise work stays on DVE: GpSimd shares SBUF ports with it, so
running both concurrently slows DVE ~2.7x (measured) and nets nothing.
"""
import heapq
import numpy as np
from contextlib import ExitStack

import concourse.bass as bass
import concourse.tile as tile
from concourse import mybir
from concourse.bass_utils import run_bass_kernel_spmd

F16 = mybir.dt.float16
F32 = mybir.dt.float32
I32 = mybir.dt.int32
AF = mybir.ActivationFunctionType
OP = mybir.AluOpType
P = 128
NCORES = 8
NWC = 20                 # windows per core
NEG_SLOPE = 0.2
RS1 = 656                # layer-1 record: 256+256+8+8 + one-hot(128)
MS1 = 520                # layer-1 message block: 256 + 8(ex) + 256
RS2 = 260                # layer-2 record: 256 + 1.0 + asrc + adst + dl
SPIN = 24                # PE warm-up matmuls (HAM releases the clock gate
                         # only after ~3.4us of sustained PE activity)


def _split_excess_waits(nc, max_waits=1):
    """This walrus build rejects instructions with >~2 sync waits; move excess
    waits onto same-engine wait-only instructions placed just before."""
    cnt = 0
    for f in nc.m.functions:
        for bb in f.blocks:
            new_insts = []
            for inst in bb.instructions:
                si = inst.sync_info
                if si is not None and si.on_wait and len(si.on_wait) > max_waits:
                    waits = list(si.on_wait)
                    extra, keep = waits[:-max_waits], waits[-max_waits:]
                    for w in extra:
                        cnt += 1
                        nop = mybir.InstNoOp(name=f"wsplit-{cnt}-{inst.name}", ins=[], outs=[])
                        nop.engine = inst.engine
                        nop.sync_info = mybir.SyncInfo(on_wait=[w], on_update=[])
                        new_insts.append(nop)
                    si.on_wait = keep
                new_insts.append(inst)
            bb.instructions = new_insts
    return cnt


def _pack_windows(dst, N, nw):
    """Greedy balance in-degree over nw windows of P slots. Returns
    win_of[N], slot_of[N], K (edge chunks per window)."""
    deg = np.bincount(dst, minlength=N)
    order = np.argsort(-deg, kind="stable")
    wload = np.zeros(nw, np.int64)
    wcnt = np.zeros(nw, np.int64)
    win_of = np.zeros(N, np.int32)
    slot_of = np.zeros(N, np.int32)
    heap = [(0, w) for w in range(nw)]
    heapq.heapify(heap)
    for n in order:
        while True:
            load, w = heapq.heappop(heap)
            if wcnt[w] < P:
                break
        win_of[n] = w
        slot_of[n] = wcnt[w]
        wcnt[w] += 1
        wload[w] += deg[n]
        if wcnt[w] < P:
            heapq.heappush(heap, (wload[w], w))
    K = int(np.ceil(wload.max() / P))
    return win_of, slot_of, K


def _kgroups(K, ng):
    """Split range(K) into ng contiguous groups for DVE/PE pipelining."""
    out = []
    base = 0
    for g in range(ng):
        n = (K - base + (ng - g) - 1) // (ng - g)
        out.append((base, base + n))
        base += n
    return out


def _spin_init(nc, const):
    a = const.tile([P, P], F16, tag="spin_a")
    nc.vector.memset(a[:], 1.0)
    b = const.tile([P, 512], F16, tag="spin_b")
    nc.vector.memset(b[:], 0.5)
    return a, b


def _spin(nc, spin_ab, pool, n):
    """Dependency-free matmuls that keep the PE HAM activity monitor busy
    (the clock gate drops to 1.2 GHz after ~3.4us of low PE activity);
    issued ahead of real matmuls so they fill operand-wait gaps."""
    a, b = spin_ab
    ps = pool.tile([P, 512], F32, tag="spin_ps")
    for i in range(n):
        nc.tensor.matmul(out=ps[:], lhsT=a[:], rhs=b[:], start=i == 0,
                         stop=i == n - 1)


def _build_a(D1, NPC):
    """h = x@W1 + fused alphas. out row = [po0(264) | po1(264)] =
    [h[0:256], asrc(8), h[256:512], adst(8)]."""
    nc = bass.Bass("TRN2", target_bir_lowering=False, debug=False, num_devices=NCORES)
    xT = nc.dram_tensor("xT", [D1, NPC], F16, kind="ExternalInput")
    W1E = nc.dram_tensor("W1E", [D1, 2 * 264], F16, kind="ExternalInput")
    tab1 = nc.dram_tensor("tab1", [NPC, 528], F16, kind="ExternalOutput")
    KB = D1 // P
    with tile.TileContext(nc) as tc:
        with ExitStack() as ctx:
            const = ctx.enter_context(tc.tile_pool(name="const", bufs=1))
            work = ctx.enter_context(tc.tile_pool(name="work", bufs=3))
            ps0 = ctx.enter_context(tc.tile_pool(name="ps0", bufs=2, space="PSUM"))
            ps1 = ctx.enter_context(tc.tile_pool(name="ps1", bufs=2, space="PSUM"))
            psW = ctx.enter_context(tc.tile_pool(name="psW", bufs=1, space="PSUM"))
            spin_ab = _spin_init(nc, const)
            _spin(nc, spin_ab, psW, SPIN)
            xsb, w0sb, w1sb = [], [], []
            for kb in range(KB):
                t = const.tile([P, NPC], F16, tag=f"x_{kb}")
                nc.sync.dma_start(out=t[:], in_=xT[kb * P:(kb + 1) * P, :])
                xsb.append(t)
                t0 = const.tile([P, 264], F16, tag=f"w0_{kb}")
                nc.sync.dma_start(out=t0[:], in_=W1E[kb * P:(kb + 1) * P, 0:264])
                w0sb.append(t0)
                t1 = const.tile([P, 264], F16, tag=f"w1_{kb}")
                nc.sync.dma_start(out=t1[:], in_=W1E[kb * P:(kb + 1) * P, 264:528])
                w1sb.append(t1)
            for t_i in range(NPC // P):
                po0 = ps0.tile([P, 264], F32, tag="po0")
                po1 = ps1.tile([P, 264], F32, tag="po1")
                for kb in range(KB):
                    lhsT = xsb[kb][:, t_i * P:(t_i + 1) * P]
                    nc.tensor.matmul(out=po0[:], lhsT=lhsT, rhs=w0sb[kb][:],
                                     start=kb == 0, stop=kb == KB - 1)
                for kb in range(KB):
                    lhsT = xsb[kb][:, t_i * P:(t_i + 1) * P]
                    nc.tensor.matmul(out=po1[:], lhsT=lhsT, rhs=w1sb[kb][:],
                                     start=kb == 0, stop=kb == KB - 1)
                stage = work.tile([P, 528], F16, tag="stage")
                nc.scalar.activation(out=stage[:, 0:264], in_=po0[:], func=AF.Copy)
                nc.vector.tensor_copy(out=stage[:, 264:528], in_=po1[:])
                nc.sync.dma_start(out=tab1[t_i * P:(t_i + 1) * P, :], in_=stage[:])
    _split_excess_waits(nc)
    return nc


def _build_b(K, D1, H1, OUTC, add_bias):
    """Layer-1 edge phase + flush into the layer-2 table.

    Elementwise work split: DVE keeps msg_lo scaling + part of the selector
    compares + the (PSUM-bound) flush; GpSimd takes msg_hi scaling, the rest
    of the compares, and the small copies. Both sliced into 4 k-groups so
    selector matmuls start early and the PE never idles long enough for the
    HAM clock gate to drop."""
    C1 = D1 // H1
    nc = bass.Bass("TRN2", target_bir_lowering=False, debug=False, num_devices=NCORES)
    EDG = nc.dram_tensor("EDG", [NWC, P, K * RS1], F16, kind="ExternalInput")
    W2E = nc.dram_tensor("W2E", [D1, OUTC + 2], F16, kind="ExternalInput")
    BB = nc.dram_tensor("BB", [P, D1], F32, kind="ExternalInput")
    tab2 = nc.dram_tensor("tab2", [NWC * P, OUTC + 2], F16, kind="ExternalOutput")
    KG = _kgroups(K, 4)
    with tile.TileContext(nc) as tc:
        with ExitStack() as ctx:
            const = ctx.enter_context(tc.tile_pool(name="const", bufs=1))
            gp = ctx.enter_context(tc.tile_pool(name="gp", bufs=3))
            mp = ctx.enter_context(tc.tile_pool(name="mp", bufs=2))
            cp = ctx.enter_context(tc.tile_pool(name="cp", bufs=2))
            sp = ctx.enter_context(tc.tile_pool(name="sp", bufs=2))
            fp = ctx.enter_context(tc.tile_pool(name="fp", bufs=2))
            st = ctx.enter_context(tc.tile_pool(name="st", bufs=2))
            ps0 = ctx.enter_context(tc.tile_pool(name="ps0", bufs=2, space="PSUM"))
            ps1 = ctx.enter_context(tc.tile_pool(name="ps1", bufs=2, space="PSUM"))
            psH = ctx.enter_context(tc.tile_pool(name="psH", bufs=2, space="PSUM"))
            psT = ctx.enter_context(tc.tile_pool(name="psT", bufs=1, space="PSUM"))
            psW = ctx.enter_context(tc.tile_pool(name="psW", bufs=1, space="PSUM"))
            spin_ab = _spin_init(nc, const)
            _spin(nc, spin_ab, psW, SPIN)

            iota_i = const.tile([P, P], I32)
            nc.gpsimd.iota(iota_i[:], pattern=[[1, P]], base=0, channel_multiplier=0)
            piota_i = const.tile([P, 1], I32)
            nc.gpsimd.iota(piota_i[:], pattern=[[0, 1]], base=0, channel_multiplier=1)
            piota_f = const.tile([P, 1], F32)
            nc.vector.tensor_copy(out=piota_f[:], in_=piota_i[:])
            iota_f = const.tile([P, P], F32)
            nc.vector.tensor_copy(out=iota_f[:], in_=iota_i[:])
            identF = const.tile([P, P], F32)
            nc.vector.tensor_tensor(out=identF[:], in0=iota_f[:],
                                    in1=piota_f[:].to_broadcast([P, P]), op=OP.is_equal)
            if add_bias:
                bb = const.tile([P, D1], F32)
                nc.sync.dma_start(out=bb[:], in_=BB[:, :])
            w2e_sb = []
            for cb in range(D1 // P):
                t = const.tile([P, OUTC + 2], F16, tag=f"w2e_{cb}")
                nc.sync.dma_start(out=t[:], in_=W2E[cb * P:(cb + 1) * P, :])
                w2e_sb.append(t)

            def flush_b(w, po0, po1):
                dr = fp.tile([P, H1], F32, tag="dr")
                nc.scalar.activation(out=dr[:], in_=po0[:, 256:264], func=AF.Copy,
                                     bias=1e-16)
                drr = fp.tile([P, H1], F32, tag="drr")
                nc.vector.reciprocal(out=drr[:], in_=dr[:])
                o1 = fp.tile([P, D1], F32, tag="o1")
                nc.vector.tensor_tensor(
                    out=o1[:, 0:256].rearrange("p (h c) -> p h c", h=4),
                    in0=po0[:, 0:256].rearrange("p (h c) -> p h c", h=4),
                    in1=drr[:, 0:4].to_broadcast([P, 4, C1]), op=OP.mult)
                nc.vector.tensor_tensor(
                    out=o1[:, 256:512].rearrange("p (h c) -> p h c", h=4),
                    in0=po1[:, 0:256].rearrange("p (h c) -> p h c", h=4),
                    in1=drr[:, 4:8].to_broadcast([P, 4, C1]), op=OP.mult)
                if add_bias:
                    nc.vector.tensor_tensor(out=o1[:], in0=o1[:], in1=bb[:], op=OP.add)
                ee = fp.tile([P, D1], F32, tag="ee")
                nc.scalar.activation(out=ee[:], in_=o1[:], func=AF.Exp)
                nc.vector.tensor_scalar(out=ee[:], in0=ee[:], scalar1=1.0,
                                        scalar2=-1.0, op0=OP.min, op1=OP.add)
                h2 = fp.tile([P, D1], F32, tag="h2")
                nc.vector.tensor_tensor(out=h2[:], in0=o1[:], in1=ee[:], op=OP.max)
                ph2 = psH.tile([P, OUTC + 2], F32, tag="ph2")
                for cb in range(D1 // P):
                    pt = psT.tile([P, P], F32, tag="pt")
                    nc.tensor.transpose(out=pt[:], in_=h2[:, cb * P:(cb + 1) * P],
                                        identity=identF[:])
                    h2t = cp.tile([P, P], F16, tag="h2t")
                    nc.scalar.activation(out=h2t[:], in_=pt[:], func=AF.Copy)
                    nc.tensor.matmul(out=ph2[:], lhsT=h2t[:], rhs=w2e_sb[cb][:],
                                     start=cb == 0, stop=cb == D1 // P - 1)
                stage = st.tile([P, OUTC + 2], F16, tag="stage")
                nc.scalar.activation(out=stage[:], in_=ph2[:], func=AF.Copy)
                nc.sync.dma_start(out=tab2[w * P:(w + 1) * P, :], in_=stage[:])

            pending = []
            for w in range(NWC):
                G = gp.tile([P, K * RS1], F16, tag="G")
                nc.sync.dma_start(out=G[:], in_=EDG[w])
                Gv = G[:].rearrange("p (k t) -> p k t", t=RS1)
                S = sp.tile([P, K * H1], F32, tag="S")
                nc.vector.tensor_tensor(
                    out=S[:].rearrange("p (k h) -> p k h", h=H1),
                    in0=Gv[:, :, 512:520], in1=Gv[:, :, 520:528], op=OP.add)
                LR = sp.tile([P, K * H1], F32, tag="LR")
                nc.scalar.activation(out=LR[:], in_=S[:], func=AF.Prelu, alpha=NEG_SLOPE)
                EX = sp.tile([P, K * H1], F16, tag="EX")
                nc.scalar.activation(out=EX[:], in_=LR[:], func=AF.Exp)
                EXv = EX[:].rearrange("p (k h) -> p k h", h=H1)

                po0 = ps0.tile([P, 264], F32, tag="po0")
                po1 = ps1.tile([P, 256], F32, tag="po1")
                for g, (k0, k1) in enumerate(KG):
                    L = k1 - k0
                    MgL = mp.tile([P, L * 264], F16, tag=f"ML{g}")
                    MgLv = MgL[:].rearrange("p (k t) -> p k t", t=264)
                    MgH = mp.tile([P, L * 256], F16, tag=f"MH{g}")
                    MgHv = MgH[:].rearrange("p (k t) -> p k t", t=264 - 8)
                    nc.scalar.activation(out=MgLv[:, :, 256:264],
                                         in_=EXv[:, k0:k1, :], func=AF.Copy)
                    nc.vector.tensor_tensor(
                        out=MgLv[:, :, 0:256].rearrange("p k (h c) -> p k h c", h=4),
                        in0=Gv[:, k0:k1, 0:256].rearrange("p k (h c) -> p k h c", h=4),
                        in1=EXv[:, k0:k1, 0:4].to_broadcast([P, L, 4, C1]), op=OP.mult)
                    nc.vector.tensor_tensor(
                        out=MgHv[:, :, :].rearrange("p k (h c) -> p k h c", h=4),
                        in0=Gv[:, k0:k1, 256:512].rearrange("p k (h c) -> p k h c", h=4),
                        in1=EXv[:, k0:k1, 4:8].to_broadcast([P, L, 4, C1]), op=OP.mult)
                    for k in range(L):
                        gk = k0 + k
                        lhsT = G[:, gk * RS1 + 528:gk * RS1 + 656]
                        nc.tensor.matmul(out=po0[:], lhsT=lhsT,
                                         rhs=MgL[:, k * 264:(k + 1) * 264],
                                         start=gk == 0, stop=gk == K - 1)
                        nc.tensor.matmul(out=po1[:], lhsT=lhsT,
                                         rhs=MgH[:, k * 256:(k + 1) * 256],
                                         start=gk == 0, stop=gk == K - 1)

                pending.append((w, po0, po1))
                if len(pending) > 1:
                    flush_b(*pending.pop(0))
            flush_b(*pending.pop(0))
    _split_excess_waits(nc)
    return nc


def _build_c(K, OUTC, add_bias):
    """Layer-2 edge phase: coefficient folded into the selector
    (CMPX = onehot * ex), raw message rows stream straight into the
    matmul; flush = divide, (+b2,) log_softmax."""
    nc = bass.Bass("TRN2", target_bir_lowering=False, debug=False, num_devices=NCORES)
    EDG = nc.dram_tensor("EDG", [NWC, P, K * RS2], F16, kind="ExternalInput")
    BB = nc.dram_tensor("BB", [P, OUTC], F32, kind="ExternalInput")
    out_t = nc.dram_tensor("out", [NWC * P, OUTC], F32, kind="ExternalOutput")
    KG = _kgroups(K, 4)
    with tile.TileContext(nc) as tc:
        with ExitStack() as ctx:
            const = ctx.enter_context(tc.tile_pool(name="const", bufs=1))
            gp = ctx.enter_context(tc.tile_pool(name="gp", bufs=3))
            cp = ctx.enter_context(tc.tile_pool(name="cp", bufs=2))
            sp = ctx.enter_context(tc.tile_pool(name="sp", bufs=2))
            fp = ctx.enter_context(tc.tile_pool(name="fp", bufs=2))
            ps0 = ctx.enter_context(tc.tile_pool(name="ps0", bufs=2, space="PSUM"))
            psW = ctx.enter_context(tc.tile_pool(name="psW", bufs=1, space="PSUM"))
            spin_ab = _spin_init(nc, const)
            _spin(nc, spin_ab, psW, SPIN)

            iota_i = const.tile([P, P], I32)
            nc.gpsimd.iota(iota_i[:], pattern=[[1, P]], base=0, channel_multiplier=0)
            iotag = const.tile([P, K * P], F16)
            for k in range(K):
                nc.gpsimd.tensor_copy(out=iotag[:, k * P:(k + 1) * P], in_=iota_i[:])
            if add_bias:
                bb = const.tile([P, OUTC], F32)
                nc.sync.dma_start(out=bb[:], in_=BB[:, :])

            def flush_c(w, po):
                dr = fp.tile([P, 1], F32, tag="dr")
                nc.scalar.activation(out=dr[:], in_=po[:, 256:257], func=AF.Copy,
                                     bias=1e-16)
                drr = fp.tile([P, 1], F32, tag="drr")
                nc.vector.reciprocal(out=drr[:], in_=dr[:])
                z = fp.tile([P, OUTC], F32, tag="z")
                nc.vector.tensor_scalar(out=z[:], in0=po[:, 0:256], scalar1=drr[:, :1],
                                        scalar2=None, op0=OP.mult)
                if add_bias:
                    nc.vector.tensor_tensor(out=z[:], in0=z[:], in1=bb[:], op=OP.add)
                ee = fp.tile([P, OUTC], F32, tag="ee")
                se = fp.tile([P, 1], F32, tag="se")
                nc.scalar.activation(out=ee[:], in_=z[:], func=AF.Exp, accum_out=se[:])
                lse = fp.tile([P, 1], F32, tag="lse")
                nc.scalar.activation(out=lse[:], in_=se[:], func=AF.Ln)
                nc.vector.tensor_scalar(out=z[:], in0=z[:], scalar1=lse[:, :1],
                                        scalar2=None, op0=OP.subtract)
                nc.sync.dma_start(out=out_t[w * P:(w + 1) * P, :], in_=z[:])

            pending = []
            for w in range(NWC):
                G = gp.tile([P, K * RS2], F16, tag="G")
                nc.sync.dma_start(out=G[:], in_=EDG[w])
                Gv = G[:].rearrange("p (k t) -> p k t", t=RS2)
                S = sp.tile([P, K], F32, tag="S")
                nc.vector.tensor_tensor(
                    out=S[:].rearrange("p (k o) -> p k o", o=1),
                    in0=Gv[:, :, 257:258], in1=Gv[:, :, 258:259], op=OP.add)
                LR = sp.tile([P, K], F32, tag="LR")
                nc.scalar.activation(out=LR[:], in_=S[:], func=AF.Prelu, alpha=NEG_SLOPE)
                EX = sp.tile([P, K], F16, tag="EX")
                nc.scalar.activation(out=EX[:], in_=LR[:], func=AF.Exp)
                EXv = EX[:].rearrange("p (k o) -> p k o", o=1)

                po = ps0.tile([P, 257], F32, tag="po")
                for g, (k0, k1) in enumerate(KG):
                    L = k1 - k0
                    _spin(nc, spin_ab, psW, 2)
                    CMPg = cp.tile([P, L * P], F16, tag=f"C{g}")
                    nc.vector.tensor_tensor(
                        out=CMPg[:].rearrange("p (k q) -> p k q", q=P),
                        in0=iotag[:, k0 * P:k1 * P].rearrange("p (k q) -> p k q", q=P),
                        in1=Gv[:, k0:k1, 259:260].rearrange("p k o -> p (k o)")
                            .to_broadcast([P, L, P]),
                        op=OP.is_equal)
                    CMXg = cp.tile([P, L * P], F16, tag=f"X{g}")
                    nc.vector.tensor_tensor(
                        out=CMXg[:].rearrange("p (k q) -> p k q", q=P),
                        in0=CMPg[:].rearrange("p (k q) -> p k q", q=P),
                        in1=EXv[:, k0:k1, :].to_broadcast([P, L, P]), op=OP.mult)
                    for k in range(L):
                        gk = k0 + k
                        nc.tensor.matmul(out=po[:], lhsT=CMXg[:, k * P:(k + 1) * P],
                                         rhs=G[:, gk * RS2:gk * RS2 + 257],
                                         start=gk == 0, stop=gk == K - 1)

                pending.append((w, po))
                if len(pending) > 1:
                    flush_c(*pending.pop(0))
            flush_c(*pending.pop(0))
    _split_excess_waits(nc)
    return nc


def kernel(x, edge_index, W1, att_src1, att_dst1, b1, W2, att_src2, att_dst2, b2):
    x = np.asarray(x, np.float32)
    edge_index = np.asarray(edge_index)
    W1d = np.asarray(W1, np.float64)
    W2d = np.asarray(W2, np.float64)
    as1 = np.asarray(att_src1, np.float64)
    ad1 = np.asarray(att_dst1, np.float64)
    as2 = np.asarray(att_src2, np.float64)
    ad2 = np.asarray(att_dst2, np.float64)
    b1 = np.asarray(b1, np.float32)
    b2 = np.asarray(b2, np.float32)
    N, D1 = x.shape
    H1, C1 = att_src1.shape
    OUTC = W2.shape[1]
    NW = NCORES * NWC
    NPC = NWC * P
    core_ids = list(range(NCORES))
    npc_in = N // NCORES

    src = np.concatenate([edge_index[0], np.arange(N)]).astype(np.int64)
    dst = np.concatenate([edge_index[1], np.arange(N)]).astype(np.int64)
    win_of, slot_of, K = _pack_windows(dst, N, NW)

    # edge -> (window, chunk, partition) in window-major stable order
    w_e = win_of[dst]
    eorder = np.argsort(w_e, kind="stable")
    sw = w_e[eorder]
    counts = np.bincount(sw, minlength=NW)
    starts = np.concatenate([[0], np.cumsum(counts)[:-1]])
    pos = np.arange(len(sw)) - starts[sw]
    k_e = (pos // P).astype(np.int64)
    p_e = (pos % P).astype(np.int64)
    s_e = src[eorder]
    d_e = dst[eorder]
    row_of_node = win_of.astype(np.int64) * P + slot_of  # global table row

    # ---- Launch A: per-node table [h_lo, asrc, h_hi, adst] ----
    asd_s = np.zeros((D1, H1))
    asd_d = np.zeros((D1, H1))
    for h in range(H1):
        asd_s[h * C1:(h + 1) * C1, h] = as1[h]
        asd_d[h * C1:(h + 1) * C1, h] = ad1[h]
    W1E = np.concatenate([W1d[:, 0:256], W1d @ asd_s, W1d[:, 256:512], W1d @ asd_d],
                         axis=1).astype(np.float16)
    nc_a = _build_a(D1, NPC)
    in_maps = []
    for c in range(NCORES):
        xo = np.zeros((NPC, D1), np.float16)
        xo[:npc_in] = x[c * npc_in:(c + 1) * npc_in].astype(np.float16)
        in_maps.append({"xT": np.ascontiguousarray(xo.T), "W1E": W1E})
    res_a = run_bass_kernel_spmd(nc_a, in_maps, core_ids)
    tab1 = np.concatenate([res_a.results[c]["tab1"][:npc_in] for c in range(NCORES)], axis=0)
    h_lo = tab1[:, 0:256]
    a_src_n = tab1[:, 256:264]
    h_hi = tab1[:, 264:520]
    a_dst_n = tab1[:, 520:528]

    # ---- Launch B: layer-1 edge phase ----
    W2E = np.concatenate([W2d, W2d @ as2.T, W2d @ ad2.T], axis=1).astype(np.float16)
    BB1 = np.tile(b1.reshape(1, D1), (P, 1))
    nc_b = _build_b(K, D1, H1, OUTC, bool(np.any(b1)))
    in_maps = []
    for c in range(NCORES):
        m = (sw >= c * NWC) & (sw < (c + 1) * NWC)
        lw, kk, pp = sw[m] - c * NWC, k_e[m], p_e[m]
        sm, dm = s_e[m], d_e[m]
        EDG = np.zeros((NWC, P, K, RS1), np.float16)
        EDG[lw, pp, kk, 0:256] = h_lo[sm]
        EDG[lw, pp, kk, 256:512] = h_hi[sm]
        EDG[lw, pp, kk, 512:520] = a_src_n[sm]
        EDG[lw, pp, kk, 520:528] = a_dst_n[dm]
        EDG[lw, pp, kk, 528 + slot_of[dm]] = 1.0
        in_maps.append({"EDG": EDG.reshape(NWC, P, K * RS1), "W2E": W2E, "BB": BB1})
    res_b = run_bass_kernel_spmd(nc_b, in_maps, core_ids)
    tab2 = np.concatenate([res_b.results[c]["tab2"] for c in range(NCORES)], axis=0)
    h2p = tab2[:, 0:256]
    a_src2_n = tab2[:, 256]
    a_dst2_n = tab2[:, 257]

    # ---- Launch C: layer-2 edge phase + log_softmax ----
    BB2 = np.tile(b2.reshape(1, OUTC), (P, 1))
    nc_c = _build_c(K, OUTC, bool(np.any(b2)))
    in_maps = []
    sr = row_of_node[s_e]
    dr_ = row_of_node[d_e]
    for c in range(NCORES):
        m = (sw >= c * NWC) & (sw < (c + 1) * NWC)
        lw, kk, pp = sw[m] - c * NWC, k_e[m], p_e[m]
        srm, drm = sr[m], dr_[m]
        EDG = np.zeros((NWC, P, K, RS2), np.float16)
        EDG[:, :, :, 259] = 255.0
        EDG[lw, pp, kk, 0:256] = h2p[srm]
        EDG[lw, pp, kk, 256] = 1.0
        EDG[lw, pp, kk, 257] = a_src2_n[srm]
        EDG[lw, pp, kk, 258] = a_dst2_n[drm]
        EDG[lw, pp, kk, 259] = slot_of[d_e[m]].astype(np.float16)
        in_maps.append({"EDG": EDG.reshape(NWC, P, K * RS2), "BB": BB2})
    res_c = run_bass_kernel_spmd(nc_c, in_maps, core_ids)
    rows = np.concatenate([res_c.results[c]["out"] for c in range(NCORES)], axis=0)
    return np.ascontiguousarray(rows[row_of_node]).astype(np.float32)


# revision 19
# speedup vs baseline: 1.0376x; 1.0000x over previous
"""2-layer GAT (GATConv x2 + log_softmax) on 8 Trainium2 NeuronCores.

fp16 streaming design (SPMD across 8 cores; host does index marshaling
between launches, device does all arithmetic):
  - Host bin-packs the 20000 nodes into 160 windows of 128 dst slots each
    (20 windows/core), balancing in-degree so every window holds ~2125
    edges -> K = ceil(max/128) = 17 chunks of 128 edges.
  - Launch A: h = x@W1 with the attention alphas fused in via host-extended
    weights [W1 | W1@asd_src | W1@asd_dst]; emits a per-node fp16 table.
  - Host pre-gathers per-edge rows (halo exchange): for each edge slot a
    656-col fp16 record [h_lo(256) | h_hi(256) | asrc(8) | adst(8) |
    one-hot dst selector(128)].
  - Launch B: per window one bulk DMA; ACT computes exp(leakyrelu(alpha)),
    DVE scales messages by the per-head coefficients (sliced per k-group so
    selector matmuls start early); PE scatter-adds messages+denominators
    into PSUM via the shipped selectors; flush: divide, ELU, @[W2 | W2@a2]
    -> layer-2 fp16 table rows.
  - Launch C: layer 2 (H=1) folds the coefficient into the selector
    (CMPX = onehot(dl) * ex), so the 256-wide messages stream raw into the
    matmul; flush = log_softmax.
All matmul operands fp16 (1 cyc/row, f32 PSUM accumulation). All big
elementwise work stays on DVE: GpSimd shares SBUF ports with it, so
running both concurrently slows DVE ~2.7x (measured) and nets nothing.
"""
import heapq
import numpy as np
from contextlib import ExitStack

import concourse.bass as bass
import concourse.tile as tile
from concourse import mybir
from concourse.bass_utils import run_bass_kernel_spmd

F16 = mybir.dt.float16
F32 = mybir.dt.float32
I32 = mybir.dt.int32
AF = mybir.ActivationFunctionType
OP = mybir.AluOpType
P = 128
NCORES = 8
NWC = 20                 # windows per core
NEG_SLOPE = 0.2
RS1 = 656                # layer-1 record: 256+256+8+8 + one-hot(128)
MS1 = 520                # layer-1 message block: 256 + 8(ex) + 256
RS2 = 260                # layer-2 record: 256 + 1.0 + asrc + adst + dl
SPIN = 24                # PE warm-up matmuls (HAM releases the clock gate
                         # only after ~3.4us of sustained PE activity)


def _split_excess_waits(nc, max_waits=1):
    """This walrus build rejects instructions with >~2 sync waits; move excess
    waits onto same-engine wait-only instructions placed just before."""
    cnt = 0
    for f in nc.m.functions:
        for bb in f.blocks:
            new_insts = []
            for inst in bb.instructions:
                si = inst.sync_info
                if si is not None and si.on_wait and len(si.on_wait) > max_waits:
                    waits = list(si.on_wait)
                    extra, keep = waits[:-max_waits], waits[-max_waits:]
                    for w in extra:
                        cnt += 1
                        nop = mybir.InstNoOp(name=f"wsplit-{cnt}-{inst.name}", ins=[], outs=[])
                        nop.engine = inst.engine
                        nop.sync_info = mybir.SyncInfo(on_wait=[w], on_update=[])
                        new_insts.append(nop)
                    si.on_wait = keep
                new_insts.append(inst)
            bb.instructions = new_insts
    return cnt


def _pack_windows(dst, N, nw):
    """Greedy balance in-degree over nw windows of P slots. Returns
    win_of[N], slot_of[N], K (edge chunks per window)."""
    deg = np.bincount(dst, minlength=N)
    order = np.argsort(-deg, kind="stable")
    wload = np.zeros(nw, np.int64)
    wcnt = np.zeros(nw, np.int64)
    win_of = np.zeros(N, np.int32)
    slot_of = np.zeros(N, np.int32)
    heap = [(0, w) for w in range(nw)]
    heapq.heapify(heap)
    for n in order:
        while True:
            load, w = heapq.heappop(heap)
            if wcnt[w] < P:
                break
        win_of[n] = w
        slot_of[n] = wcnt[w]
        wcnt[w] += 1
        wload[w] += deg[n]
        if wcnt[w] < P:
            heapq.heappush(heap, (wload[w], w))
    K = int(np.ceil(wload.max() / P))
    return win_of, slot_of, K


def _kgroups(K, ng):
    """Split range(K) into ng contiguous groups for DVE/PE pipelining."""
    out = []
    base = 0
    for g in range(ng):
        n = (K - base + (ng - g) - 1) // (ng - g)
        out.append((base, base + n))
        base += n
    return out


def _spin_init(nc, const):
    a = const.tile([P, P], F16, tag="spin_a")
    nc.vector.memset(a[:], 1.0)
    b = const.tile([P, 512], F16, tag="spin_b")
    nc.vector.memset(b[:], 0.5)
    return a, b


def _spin(nc, spin_ab, pool, n):
    """Dependency-free matmuls that keep the PE HAM activity monitor busy
    (the clock gate drops to 1.2 GHz after ~3.4us of low PE activity);
    issued ahead of real matmuls so they fill operand-wait gaps."""
    a, b = spin_ab
    ps = pool.tile([P, 512], F32, tag="spin_ps")
    for i in range(n):
        nc.tensor.matmul(out=ps[:], lhsT=a[:], rhs=b[:], start=i == 0,
                         stop=i == n - 1)


def _build_a(D1, NPC):
    """h = x@W1 + fused alphas. out row = [po0(264) | po1(264)] =
    [h[0:256], asrc(8), h[256:512], adst(8)]."""
    nc = bass.Bass("TRN2", target_bir_lowering=False, debug=False, num_devices=NCORES)
    xT = nc.dram_tensor("xT", [D1, NPC], F16, kind="ExternalInput")
    W1E = nc.dram_tensor("W1E", [D1, 2 * 264], F16, kind="ExternalInput")
    tab1 = nc.dram_tensor("tab1", [NPC, 528], F16, kind="ExternalOutput")
    KB = D1 // P
    with tile.TileContext(nc) as tc:
        with ExitStack() as ctx:
            const = ctx.enter_context(tc.tile_pool(name="const", bufs=1))
            work = ctx.enter_context(tc.tile_pool(name="work", bufs=3))
            ps0 = ctx.enter_context(tc.tile_pool(name="ps0", bufs=2, space="PSUM"))
            ps1 = ctx.enter_context(tc.tile_pool(name="ps1", bufs=2, space="PSUM"))
            psW = ctx.enter_context(tc.tile_pool(name="psW", bufs=1, space="PSUM"))
            spin_ab = _spin_init(nc, const)
            _spin(nc, spin_ab, psW, SPIN)
            xsb, w0sb, w1sb = [], [], []
            for kb in range(KB):
                t = const.tile([P, NPC], F16, tag=f"x_{kb}")
                nc.sync.dma_start(out=t[:], in_=xT[kb * P:(kb + 1) * P, :])
                xsb.append(t)
                t0 = const.tile([P, 264], F16, tag=f"w0_{kb}")
                nc.sync.dma_start(out=t0[:], in_=W1E[kb * P:(kb + 1) * P, 0:264])
                w0sb.append(t0)
                t1 = const.tile([P, 264], F16, tag=f"w1_{kb}")
                nc.sync.dma_start(out=t1[:], in_=W1E[kb * P:(kb + 1) * P, 264:528])
                w1sb.append(t1)
            for t_i in range(NPC // P):
                po0 = ps0.tile([P, 264], F32, tag="po0")
                po1 = ps1.tile([P, 264], F32, tag="po1")
                for kb in range(KB):
                    lhsT = xsb[kb][:, t_i * P:(t_i + 1) * P]
                    nc.tensor.matmul(out=po0[:], lhsT=lhsT, rhs=w0sb[kb][:],
                                     start=kb == 0, stop=kb == KB - 1)
                for kb in range(KB):
                    lhsT = xsb[kb][:, t_i * P:(t_i + 1) * P]
                    nc.tensor.matmul(out=po1[:], lhsT=lhsT, rhs=w1sb[kb][:],
                                     start=kb == 0, stop=kb == KB - 1)
                stage = work.tile([P, 528], F16, tag="stage")
                nc.scalar.activation(out=stage[:, 0:264], in_=po0[:], func=AF.Copy)
                nc.vector.tensor_copy(out=stage[:, 264:528], in_=po1[:])
                nc.sync.dma_start(out=tab1[t_i * P:(t_i + 1) * P, :], in_=stage[:])
    _split_excess_waits(nc)
    return nc


def _build_b(K, D1, H1, OUTC, add_bias):
    """Layer-1 edge phase + flush into the layer-2 table.

    Elementwise work split: DVE keeps msg_lo scaling + part of the selector
    compares + the (PSUM-bound) flush; GpSimd takes msg_hi scaling, the rest
    of the compares, and the small copies. Both sliced into 4 k-groups so
    selector matmuls start early and the PE never idles long enough for the
    HAM clock gate to drop."""
    C1 = D1 // H1
    nc = bass.Bass("TRN2", target_bir_lowering=False, debug=False, num_devices=NCORES)
    EDG = nc.dram_tensor("EDG", [NWC, P, K * RS1], F16, kind="ExternalInput")
    W2E = nc.dram_tensor("W2E", [D1, OUTC + 2], F16, kind="ExternalInput")
    BB = nc.dram_tensor("BB", [P, D1], F32, kind="ExternalInput")
    tab2 = nc.dram_tensor("tab2", [NWC * P, OUTC + 2], F16, kind="ExternalOutput")
    KG = _kgroups(K, 4)
    with tile.TileContext(nc) as tc:
        with ExitStack() as ctx:
            const = ctx.enter_context(tc.tile_pool(name="const", bufs=1))
            gp = ctx.enter_context(tc.tile_pool(name="gp", bufs=4))
            mp = ctx.enter_context(tc.tile_pool(name="mp", bufs=3))
            cp = ctx.enter_context(tc.tile_pool(name="cp", bufs=2))
            sp = ctx.enter_context(tc.tile_pool(name="sp", bufs=3))
            fp = ctx.enter_context(tc.tile_pool(name="fp", bufs=2))
            st = ctx.enter_context(tc.tile_pool(name="st", bufs=2))
            ps0 = ctx.enter_context(tc.tile_pool(name="ps0", bufs=2, space="PSUM"))
            ps1 = ctx.enter_context(tc.tile_pool(name="ps1", bufs=2, space="PSUM"))
            psH = ctx.enter_context(tc.tile_pool(name="psH", bufs=2, space="PSUM"))
            psT = ctx.enter_context(tc.tile_pool(name="psT", bufs=1, space="PSUM"))
            psW = ctx.enter_context(tc.tile_pool(name="psW", bufs=1, space="PSUM"))
            spin_ab = _spin_init(nc, const)
            _spin(nc, spin_ab, psW, SPIN)

            iota_i = const.tile([P, P], I32)
            nc.gpsimd.iota(iota_i[:], pattern=[[1, P]], base=0, channel_multiplier=0)
            piota_i = const.tile([P, 1], I32)
            nc.gpsimd.iota(piota_i[:], pattern=[[0, 1]], base=0, channel_multiplier=1)
            piota_f = const.tile([P, 1], F32)
            nc.vector.tensor_copy(out=piota_f[:], in_=piota_i[:])
            iota_f = const.tile([P, P], F32)
            nc.vector.tensor_copy(out=iota_f[:], in_=iota_i[:])
            identF = const.tile([P, P], F32)
            nc.vector.tensor_tensor(out=identF[:], in0=iota_f[:],
                                    in1=piota_f[:].to_broadcast([P, P]), op=OP.is_equal)
            if add_bias:
                bb = const.tile([P, D1], F32)
                nc.sync.dma_start(out=bb[:], in_=BB[:, :])
            w2e_sb = []
            for cb in range(D1 // P):
                t = const.tile([P, OUTC + 2], F16, tag=f"w2e_{cb}")
                nc.sync.dma_start(out=t[:], in_=W2E[cb * P:(cb + 1) * P, :])
                w2e_sb.append(t)

            def flush_b(w, po0, po1):
                dr = fp.tile([P, H1], F32, tag="dr")
                nc.scalar.activation(out=dr[:], in_=po0[:, 256:264], func=AF.Copy,
                                     bias=1e-16)
                drr = fp.tile([P, H1], F32, tag="drr")
                nc.vector.reciprocal(out=drr[:], in_=dr[:])
                o1 = fp.tile([P, D1], F32, tag="o1")
                nc.vector.tensor_tensor(
                    out=o1[:, 0:256].rearrange("p (h c) -> p h c", h=4),
                    in0=po0[:, 0:256].rearrange("p (h c) -> p h c", h=4),
                    in1=drr[:, 0:4].to_broadcast([P, 4, C1]), op=OP.mult)
                nc.vector.tensor_tensor(
                    out=o1[:, 256:512].rearrange("p (h c) -> p h c", h=4),
                    in0=po1[:, 0:256].rearrange("p (h c) -> p h c", h=4),
                    in1=drr[:, 4:8].to_broadcast([P, 4, C1]), op=OP.mult)
                if add_bias:
                    nc.vector.tensor_tensor(out=o1[:], in0=o1[:], in1=bb[:], op=OP.add)
                ee = fp.tile([P, D1], F32, tag="ee")
                nc.scalar.activation(out=ee[:], in_=o1[:], func=AF.Exp)
                nc.vector.tensor_scalar(out=ee[:], in0=ee[:], scalar1=1.0,
                                        scalar2=-1.0, op0=OP.min, op1=OP.add)
                h2 = fp.tile([P, D1], F32, tag="h2")
                nc.vector.tensor_tensor(out=h2[:], in0=o1[:], in1=ee[:], op=OP.max)
                ph2 = psH.tile([P, OUTC + 2], F32, tag="ph2")
                for cb in range(D1 // P):
                    pt = psT.tile([P, P], F32, tag="pt")
                    nc.tensor.transpose(out=pt[:], in_=h2[:, cb * P:(cb + 1) * P],
                                        identity=identF[:])
                    h2t = cp.tile([P, P], F16, tag="h2t")
                    nc.scalar.activation(out=h2t[:], in_=pt[:], func=AF.Copy)
                    nc.tensor.matmul(out=ph2[:], lhsT=h2t[:], rhs=w2e_sb[cb][:],
                                     start=cb == 0, stop=cb == D1 // P - 1)
                stage = st.tile([P, OUTC + 2], F16, tag="stage")
                nc.scalar.activation(out=stage[:], in_=ph2[:], func=AF.Copy)
                nc.sync.dma_start(out=tab2[w * P:(w + 1) * P, :], in_=stage[:])

            pending = []
            for w in range(NWC):
                G = gp.tile([P, K * RS1], F16, tag="G")
                nc.sync.dma_start(out=G[:], in_=EDG[w])
                Gv = G[:].rearrange("p (k t) -> p k t", t=RS1)
                S = sp.tile([P, K * H1], F32, tag="S")
                nc.vector.tensor_tensor(
                    out=S[:].rearrange("p (k h) -> p k h", h=H1),
                    in0=Gv[:, :, 512:520], in1=Gv[:, :, 520:528], op=OP.add)
                LR = sp.tile([P, K * H1], F32, tag="LR")
                nc.scalar.activation(out=LR[:], in_=S[:], func=AF.Prelu, alpha=NEG_SLOPE)
                EX = sp.tile([P, K * H1], F16, tag="EX")
                nc.scalar.activation(out=EX[:], in_=LR[:], func=AF.Exp)
                EXv = EX[:].rearrange("p (k h) -> p k h", h=H1)

                po0 = ps0.tile([P, 264], F32, tag="po0")
                po1 = ps1.tile([P, 256], F32, tag="po1")
                for g, (k0, k1) in enumerate(KG):
                    L = k1 - k0
                    MgL = mp.tile([P, L * 264], F16, tag=f"ML{g}")
                    MgLv = MgL[:].rearrange("p (k t) -> p k t", t=264)
                    MgH = mp.tile([P, L * 256], F16, tag=f"MH{g}")
                    MgHv = MgH[:].rearrange("p (k t) -> p k t", t=264 - 8)
                    nc.scalar.activation(out=MgLv[:, :, 256:264],
                                         in_=EXv[:, k0:k1, :], func=AF.Copy)
                    nc.vector.tensor_tensor(
                        out=MgLv[:, :, 0:256].rearrange("p k (h c) -> p k h c", h=4),
                        in0=Gv[:, k0:k1, 0:256].rearrange("p k (h c) -> p k h c", h=4),
                        in1=EXv[:, k0:k1, 0:4].to_broadcast([P, L, 4, C1]), op=OP.mult)
                    nc.vector.tensor_tensor(
                        out=MgHv[:, :, :].rearrange("p k (h c) -> p k h c", h=4),
                        in0=Gv[:, k0:k1, 256:512].rearrange("p k (h c) -> p k h c", h=4),
                        in1=EXv[:, k0:k1, 4:8].to_broadcast([P, L, 4, C1]), op=OP.mult)
                    for k in range(L):
                        gk = k0 + k
                        lhsT = G[:, gk * RS1 + 528:gk * RS1 + 656]
                        nc.tensor.matmul(out=po0[:], lhsT=lhsT,
                                         rhs=MgL[:, k * 264:(k + 1) * 264],
                                         start=gk == 0, stop=gk == K - 1)
                        nc.tensor.matmul(out=po1[:], lhsT=lhsT,
                                         rhs=MgH[:, k * 256:(k + 1) * 256],
                                         start=gk == 0, stop=gk == K - 1)

                pending.append((w, po0, po1))
                if len(pending) > 1:
                    flush_b(*pending.pop(0))
            flush_b(*pending.pop(0))
    _split_excess_waits(nc)
    return nc


def _build_c(K, OUTC, add_bias):
    """Layer-2 edge phase: coefficient folded into the selector
    (CMPX = onehot * ex), raw message rows stream straight into the
    matmul; flush = divide, (+b2,) log_softmax."""
    nc = bass.Bass("TRN2", target_bir_lowering=False, debug=False, num_devices=NCORES)
    EDG = nc.dram_tensor("EDG", [NWC, P, K * RS2], F16, kind="ExternalInput")
    BB = nc.dram_tensor("BB", [P, OUTC], F32, kind="ExternalInput")
    out_t = nc.dram_tensor("out", [NWC * P, OUTC], F32, kind="ExternalOutput")
    KG = _kgroups(K, 4)
    with tile.TileContext(nc) as tc:
        with ExitStack() as ctx:
            const = ctx.enter_context(tc.tile_pool(name="const", bufs=1))
            gp = ctx.enter_context(tc.tile_pool(name="gp", bufs=4))
            cp = ctx.enter_context(tc.tile_pool(name="cp", bufs=3))
            sp = ctx.enter_context(tc.tile_pool(name="sp", bufs=3))
            fp = ctx.enter_context(tc.tile_pool(name="fp", bufs=2))
            ps0 = ctx.enter_context(tc.tile_pool(name="ps0", bufs=2, space="PSUM"))
            psW = ctx.enter_context(tc.tile_pool(name="psW", bufs=1, space="PSUM"))
            spin_ab = _spin_init(nc, const)
            _spin(nc, spin_ab, psW, SPIN)

            iota_i = const.tile([P, P], I32)
            nc.gpsimd.iota(iota_i[:], pattern=[[1, P]], base=0, channel_multiplier=0)
            iotag = const.tile([P, K * P], F16)
            for k in range(K):
                nc.gpsimd.tensor_copy(out=iotag[:, k * P:(k + 1) * P], in_=iota_i[:])
            if add_bias:
                bb = const.tile([P, OUTC], F32)
                nc.sync.dma_start(out=bb[:], in_=BB[:, :])

            def flush_c(w, po):
                dr = fp.tile([P, 1], F32, tag="dr")
                nc.scalar.activation(out=dr[:], in_=po[:, 256:257], func=AF.Copy,
                                     bias=1e-16)
                drr = fp.tile([P, 1], F32, tag="drr")
                nc.vector.reciprocal(out=drr[:], in_=dr[:])
                z = fp.tile([P, OUTC], F32, tag="z")
                nc.vector.tensor_scalar(out=z[:], in0=po[:, 0:256], scalar1=drr[:, :1],
                                        scalar2=None, op0=OP.mult)
                if add_bias:
                    nc.vector.tensor_tensor(out=z[:], in0=z[:], in1=bb[:], op=OP.add)
                ee = fp.tile([P, OUTC], F32, tag="ee")
                se = fp.tile([P, 1], F32, tag="se")
                nc.scalar.activation(out=ee[:], in_=z[:], func=AF.Exp, accum_out=se[:])
                lse = fp.tile([P, 1], F32, tag="lse")
                nc.scalar.activation(out=lse[:], in_=se[:], func=AF.Ln)
                nc.vector.tensor_scalar(out=z[:], in0=z[:], scalar1=lse[:, :1],
                                        scalar2=None, op0=OP.subtract)
                nc.sync.dma_start(out=out_t[w * P:(w + 1) * P, :], in_=z[:])

            pending = []
            for w in range(NWC):
                G = gp.tile([P, K * RS2], F16, tag="G")
                nc.sync.dma_start(out=G[:], in_=EDG[w])
                Gv = G[:].rearrange("p (k t) -> p k t", t=RS2)
                S = sp.tile([P, K], F32, tag="S")
                nc.vector.tensor_tensor(
                    out=S[:].rearrange("p (k o) -> p k o", o=1),
                    in0=Gv[:, :, 257:258], in1=Gv[:, :, 258:259], op=OP.add)
                LR = sp.tile([P, K], F32, tag="LR")
                nc.scalar.activation(out=LR[:], in_=S[:], func=AF.Prelu, alpha=NEG_SLOPE)
                EX = sp.tile([P, K], F16, tag="EX")
                nc.scalar.activation(out=EX[:], in_=LR[:], func=AF.Exp)
                EXv = EX[:].rearrange("p (k o) -> p k o", o=1)

                po = ps0.tile([P, 257], F32, tag="po")
                for g, (k0, k1) in enumerate(KG):
                    L = k1 - k0
                    _spin(nc, spin_ab, psW, 2)
                    CMPg = cp.tile([P, L * P], F16, tag=f"C{g}")
                    nc.vector.tensor_tensor(
                        out=CMPg[:].rearrange("p (k q) -> p k q", q=P),
                        in0=iotag[:, k0 * P:k1 * P].rearrange("p (k q) -> p k q", q=P),
                        in1=Gv[:, k0:k1, 259:260].rearrange("p k o -> p (k o)")
                            .to_broadcast([P, L, P]),
                        op=OP.is_equal)
                    CMXg = cp.tile([P, L * P], F16, tag=f"X{g}")
                    nc.vector.tensor_tensor(
                        out=CMXg[:].rearrange("p (k q) -> p k q", q=P),
                        in0=CMPg[:].rearrange("p (k q) -> p k q", q=P),
                        in1=EXv[:, k0:k1, :].to_broadcast([P, L, P]), op=OP.mult)
                    for k in range(L):
                        gk = k0 + k
                        nc.tensor.matmul(out=po[:], lhsT=CMXg[:, k * P:(k + 1) * P],
                                         rhs=G[:, gk * RS2:gk * RS2 + 257],
                                         start=gk == 0, stop=gk == K - 1)

                pending.append((w, po))
                if len(pending) > 1:
                    flush_c(*pending.pop(0))
            flush_c(*pending.pop(0))
    _split_excess_waits(nc)
    return nc


def kernel(x, edge_index, W1, att_src1, att_dst1, b1, W2, att_src2, att_dst2, b2):
    x = np.asarray(x, np.float32)
    edge_index = np.asarray(edge_index)
    W1d = np.asarray(W1, np.float64)
    W2d = np.asarray(W2, np.float64)
    as1 = np.asarray(att_src1, np.float64)
    ad1 = np.asarray(att_dst1, np.float64)
    as2 = np.asarray(att_src2, np.float64)
    ad2 = np.asarray(att_dst2, np.float64)
    b1 = np.asarray(b1, np.float32)
    b2 = np.asarray(b2, np.float32)
    N, D1 = x.shape
    H1, C1 = att_src1.shape
    OUTC = W2.shape[1]
    NW = NCORES * NWC
    NPC = NWC * P
    core_ids = list(range(NCORES))
    npc_in = N // NCORES

    src = np.concatenate([edge_index[0], np.arange(N)]).astype(np.int64)
    dst = np.concatenate([edge_index[1], np.arange(N)]).astype(np.int64)
    win_of, slot_of, K = _pack_windows(dst, N, NW)

    # edge -> (window, chunk, partition) in window-major stable order
    w_e = win_of[dst]
    eorder = np.argsort(w_e, kind="stable")
    sw = w_e[eorder]
    counts = np.bincount(sw, minlength=NW)
    starts = np.concatenate([[0], np.cumsum(counts)[:-1]])
    pos = np.arange(len(sw)) - starts[sw]
    k_e = (pos // P).astype(np.int64)
    p_e = (pos % P).astype(np.int64)
    s_e = src[eorder]
    d_e = dst[eorder]
    row_of_node = win_of.astype(np.int64) * P + slot_of  # global table row

    # ---- Launch A: per-node table [h_lo, asrc, h_hi, adst] ----
    asd_s = np.zeros((D1, H1))
    asd_d = np.zeros((D1, H1))
    for h in range(H1):
        asd_s[h * C1:(h + 1) * C1, h] = as1[h]
        asd_d[h * C1:(h + 1) * C1, h] = ad1[h]
    W1E = np.concatenate([W1d[:, 0:256], W1d @ asd_s, W1d[:, 256:512], W1d @ asd_d],
                         axis=1).astype(np.float16)
    nc_a = _build_a(D1, NPC)
    in_maps = []
    for c in range(NCORES):
        xo = np.zeros((NPC, D1), np.float16)
        xo[:npc_in] = x[c * npc_in:(c + 1) * npc_in].astype(np.float16)
        in_maps.append({"xT": np.ascontiguousarray(xo.T), "W1E": W1E})
    res_a = run_bass_kernel_spmd(nc_a, in_maps, core_ids)
    tab1 = np.concatenate([res_a.results[c]["tab1"][:npc_in] for c in range(NCORES)], axis=0)
    h_lo = tab1[:, 0:256]
    a_src_n = tab1[:, 256:264]
    h_hi = tab1[:, 264:520]
    a_dst_n = tab1[:, 520:528]

    # ---- Launch B: layer-1 edge phase ----
    W2E = np.concatenate([W2d, W2d @ as2.T, W2d @ ad2.T], axis=1).astype(np.float16)
    BB1 = np.tile(b1.reshape(1, D1), (P, 1))
    nc_b = _build_b(K, D1, H1, OUTC, bool(np.any(b1)))
    in_maps = []
    for c in range(NCORES):
        m = (sw >= c * NWC) & (sw < (c + 1) * NWC)
        lw, kk, pp = sw[m] - c * NWC, k_e[m], p_e[m]
        sm, dm = s_e[m], d_e[m]
        EDG = np.zeros((NWC, P, K, RS1), np.float16)
        EDG[lw, pp, kk, 0:256] = h_lo[sm]
        EDG[lw, pp, kk, 256:512] = h_hi[sm]
        EDG[lw, pp, kk, 512:520] = a_src_n[sm]
        EDG[lw, pp, kk, 520:528] = a_dst_n[dm]
        EDG[lw, pp, kk, 528 + slot_of[dm]] = 1.0
        in_maps.append({"EDG": EDG.reshape(NWC, P, K * RS1), "W2E": W2E, "BB": BB1})
    res_b = run_bass_kernel_spmd(nc_b, in_maps, core_ids)
    tab2 = np.concatenate([res_b.results[c]["tab2"] for c in range(NCORES)], axis=0)
    h2p = tab2[:, 0:256]
    a_src2_n = tab2[:, 256]
    a_dst2_n = tab2[:, 257]

    # ---- Launch C: layer-2 edge phase + log_softmax ----
    BB2 = np.tile(b2.reshape(1, OUTC), (P, 1))
    nc_c = _build_c(K, OUTC, bool(np.any(b2)))
    in_maps = []
    sr = row_of_node[s_e]
    dr_ = row_of_node[d_e]
    for c in range(NCORES):
        m = (sw >= c * NWC) & (sw < (c + 1) * NWC)
        lw, kk, pp = sw[m] - c * NWC, k_e[m], p_e[m]
        srm, drm = sr[m], dr_[m]
        EDG = np.zeros((NWC, P, K, RS2), np.float16)
        EDG[:, :, :, 259] = 255.0
        EDG[lw, pp, kk, 0:256] = h2p[srm]
        EDG[lw, pp, kk, 256] = 1.0
        EDG[lw, pp, kk, 257] = a_src2_n[srm]
        EDG[lw, pp, kk, 258] = a_dst2_n[drm]
        EDG[lw, pp, kk, 259] = slot_of[d_e[m]].astype(np.float16)
        in_maps.append({"EDG": EDG.reshape(NWC, P, K * RS2), "BB": BB2})
    res_c = run_bass_kernel_spmd(nc_c, in_maps, core_ids)
    rows = np.concatenate([res_c.results[c]["out"] for c in range(NCORES)], axis=0)
    return np.ascontiguousarray(rows[row_of_node]).astype(np.float32)


# revision 20
# speedup vs baseline: 1.0451x; 1.0072x over previous
"""2-layer GAT (GATConv x2 + log_softmax) on 8 Trainium2 NeuronCores.

fp16 streaming design (SPMD across 8 cores; host does index marshaling
between launches, device does all arithmetic):
  - Host bin-packs the 20000 nodes into 160 windows of 128 dst slots each
    (20 windows/core), balancing in-degree so every window holds ~2125
    edges -> K = ceil(max/128) = 17 chunks of 128 edges.
  - Launch A: h = x@W1 with the attention alphas fused in via host-extended
    weights [W1 | W1@asd_src | W1@asd_dst]; emits a per-node fp16 table.
  - Host pre-gathers per-edge rows (halo exchange): for each edge slot a
    656-col fp16 record [h_lo(256) | h_hi(256) | asrc(8) | adst(8) |
    one-hot dst selector(128)].
  - Launch B: per window one bulk DMA; ACT computes exp(leakyrelu(alpha)),
    DVE scales messages by the per-head coefficients (sliced per k-group so
    selector matmuls start early); PE scatter-adds messages+denominators
    into PSUM via the shipped selectors; flush: divide, ELU, @[W2 | W2@a2]
    -> layer-2 fp16 table rows.
  - Launch C: layer 2 (H=1) folds the coefficient into the selector
    (CMPX = onehot(dl) * ex), so the 256-wide messages stream raw into the
    matmul; flush = log_softmax.
All matmul operands fp16 (1 cyc/row, f32 PSUM accumulation). All big
elementwise work stays on DVE: GpSimd shares SBUF ports with it, so
running both concurrently slows DVE ~2.7x (measured) and nets nothing.
"""
import heapq
import numpy as np
from contextlib import ExitStack

import concourse.bass as bass
import concourse.tile as tile
from concourse import mybir
from concourse.bass_utils import run_bass_kernel_spmd

F16 = mybir.dt.float16
F32 = mybir.dt.float32
I32 = mybir.dt.int32
AF = mybir.ActivationFunctionType
OP = mybir.AluOpType
P = 128
NCORES = 8
NWC = 20                 # windows per core
NEG_SLOPE = 0.2
RS1 = 656                # layer-1 record: 256+256+8+8 + one-hot(128)
MS1 = 520                # layer-1 message block: 256 + 8(ex) + 256
RS2 = 260                # layer-2 record: 256 + 1.0 + asrc + adst + dl
SPIN = 24                # PE warm-up matmuls (HAM releases the clock gate
                         # only after ~3.4us of sustained PE activity)


def _split_excess_waits(nc, max_waits=1):
    """This walrus build rejects instructions with >~2 sync waits; move excess
    waits onto same-engine wait-only instructions placed just before."""
    cnt = 0
    for f in nc.m.functions:
        for bb in f.blocks:
            new_insts = []
            for inst in bb.instructions:
                si = inst.sync_info
                if si is not None and si.on_wait and len(si.on_wait) > max_waits:
                    waits = list(si.on_wait)
                    extra, keep = waits[:-max_waits], waits[-max_waits:]
                    for w in extra:
                        cnt += 1
                        nop = mybir.InstNoOp(name=f"wsplit-{cnt}-{inst.name}", ins=[], outs=[])
                        nop.engine = inst.engine
                        nop.sync_info = mybir.SyncInfo(on_wait=[w], on_update=[])
                        new_insts.append(nop)
                    si.on_wait = keep
                new_insts.append(inst)
            bb.instructions = new_insts
    return cnt


def _pack_windows(dst, N, nw):
    """Greedy balance in-degree over nw windows of P slots. Returns
    win_of[N], slot_of[N], K (edge chunks per window)."""
    deg = np.bincount(dst, minlength=N)
    order = np.argsort(-deg, kind="stable")
    wload = np.zeros(nw, np.int64)
    wcnt = np.zeros(nw, np.int64)
    win_of = np.zeros(N, np.int32)
    slot_of = np.zeros(N, np.int32)
    heap = [(0, w) for w in range(nw)]
    heapq.heapify(heap)
    for n in order:
        while True:
            load, w = heapq.heappop(heap)
            if wcnt[w] < P:
                break
        win_of[n] = w
        slot_of[n] = wcnt[w]
        wcnt[w] += 1
        wload[w] += deg[n]
        if wcnt[w] < P:
            heapq.heappush(heap, (wload[w], w))
    K = int(np.ceil(wload.max() / P))
    return win_of, slot_of, K


def _kgroups(K, ng):
    """Split range(K) into ng contiguous groups for DVE/PE pipelining."""
    out = []
    base = 0
    for g in range(ng):
        n = (K - base + (ng - g) - 1) // (ng - g)
        out.append((base, base + n))
        base += n
    return out


def _spin_init(nc, const):
    a = const.tile([P, P], F16, tag="spin_a")
    nc.vector.memset(a[:], 1.0)
    b = const.tile([P, 512], F16, tag="spin_b")
    nc.vector.memset(b[:], 0.5)
    return a, b


def _spin(nc, spin_ab, pool, n):
    """Dependency-free matmuls that keep the PE HAM activity monitor busy
    (the clock gate drops to 1.2 GHz after ~3.4us of low PE activity);
    issued ahead of real matmuls so they fill operand-wait gaps."""
    a, b = spin_ab
    ps = pool.tile([P, 512], F32, tag="spin_ps")
    for i in range(n):
        nc.tensor.matmul(out=ps[:], lhsT=a[:], rhs=b[:], start=i == 0,
                         stop=i == n - 1)


def _build_a(D1, NPC):
    """h = x@W1 + fused alphas. out row = [po0(264) | po1(264)] =
    [h[0:256], asrc(8), h[256:512], adst(8)]."""
    nc = bass.Bass("TRN2", target_bir_lowering=False, debug=False, num_devices=NCORES)
    xT = nc.dram_tensor("xT", [D1, NPC], F16, kind="ExternalInput")
    W1E = nc.dram_tensor("W1E", [D1, 2 * 264], F16, kind="ExternalInput")
    tab1 = nc.dram_tensor("tab1", [NPC, 528], F16, kind="ExternalOutput")
    KB = D1 // P
    with tile.TileContext(nc) as tc:
        with ExitStack() as ctx:
            const = ctx.enter_context(tc.tile_pool(name="const", bufs=1))
            work = ctx.enter_context(tc.tile_pool(name="work", bufs=3))
            ps0 = ctx.enter_context(tc.tile_pool(name="ps0", bufs=2, space="PSUM"))
            ps1 = ctx.enter_context(tc.tile_pool(name="ps1", bufs=2, space="PSUM"))
            psW = ctx.enter_context(tc.tile_pool(name="psW", bufs=1, space="PSUM"))
            spin_ab = _spin_init(nc, const)
            _spin(nc, spin_ab, psW, SPIN)
            xsb, w0sb, w1sb = [], [], []
            for kb in range(KB):
                t = const.tile([P, NPC], F16, tag=f"x_{kb}")
                nc.sync.dma_start(out=t[:], in_=xT[kb * P:(kb + 1) * P, :])
                xsb.append(t)
                t0 = const.tile([P, 264], F16, tag=f"w0_{kb}")
                nc.sync.dma_start(out=t0[:], in_=W1E[kb * P:(kb + 1) * P, 0:264])
                w0sb.append(t0)
                t1 = const.tile([P, 264], F16, tag=f"w1_{kb}")
                nc.sync.dma_start(out=t1[:], in_=W1E[kb * P:(kb + 1) * P, 264:528])
                w1sb.append(t1)
            for t_i in range(NPC // P):
                po0 = ps0.tile([P, 264], F32, tag="po0")
                po1 = ps1.tile([P, 264], F32, tag="po1")
                for kb in range(KB):
                    lhsT = xsb[kb][:, t_i * P:(t_i + 1) * P]
                    nc.tensor.matmul(out=po0[:], lhsT=lhsT, rhs=w0sb[kb][:],
                                     start=kb == 0, stop=kb == KB - 1)
                for kb in range(KB):
                    lhsT = xsb[kb][:, t_i * P:(t_i + 1) * P]
                    nc.tensor.matmul(out=po1[:], lhsT=lhsT, rhs=w1sb[kb][:],
                                     start=kb == 0, stop=kb == KB - 1)
                stage = work.tile([P, 528], F16, tag="stage")
                nc.scalar.activation(out=stage[:, 0:264], in_=po0[:], func=AF.Copy)
                nc.vector.tensor_copy(out=stage[:, 264:528], in_=po1[:])
                nc.sync.dma_start(out=tab1[t_i * P:(t_i + 1) * P, :], in_=stage[:])
    _split_excess_waits(nc)
    return nc


def _build_b(K, D1, H1, OUTC, add_bias):
    """Layer-1 edge phase + flush into the layer-2 table.

    Elementwise work split: DVE keeps msg_lo scaling + part of the selector
    compares + the (PSUM-bound) flush; GpSimd takes msg_hi scaling, the rest
    of the compares, and the small copies. Both sliced into 4 k-groups so
    selector matmuls start early and the PE never idles long enough for the
    HAM clock gate to drop."""
    C1 = D1 // H1
    nc = bass.Bass("TRN2", target_bir_lowering=False, debug=False, num_devices=NCORES)
    EDG = nc.dram_tensor("EDG", [NWC, P, K * RS1], F16, kind="ExternalInput")
    W2E = nc.dram_tensor("W2E", [D1, OUTC + 2], F16, kind="ExternalInput")
    BB = nc.dram_tensor("BB", [P, D1], F32, kind="ExternalInput")
    tab2 = nc.dram_tensor("tab2", [NWC * P, OUTC + 2], F16, kind="ExternalOutput")
    KG = _kgroups(K, 4)
    with tile.TileContext(nc) as tc:
        with ExitStack() as ctx:
            const = ctx.enter_context(tc.tile_pool(name="const", bufs=1))
            gp = ctx.enter_context(tc.tile_pool(name="gp", bufs=4))
            mp = ctx.enter_context(tc.tile_pool(name="mp", bufs=3))
            cp = ctx.enter_context(tc.tile_pool(name="cp", bufs=2))
            sp = ctx.enter_context(tc.tile_pool(name="sp", bufs=3))
            fp = ctx.enter_context(tc.tile_pool(name="fp", bufs=2))
            st = ctx.enter_context(tc.tile_pool(name="st", bufs=2))
            ps0 = ctx.enter_context(tc.tile_pool(name="ps0", bufs=2, space="PSUM"))
            ps1 = ctx.enter_context(tc.tile_pool(name="ps1", bufs=2, space="PSUM"))
            psD = ctx.enter_context(tc.tile_pool(name="psD", bufs=2, space="PSUM"))
            psH = ctx.enter_context(tc.tile_pool(name="psH", bufs=1, space="PSUM"))
            psT = ctx.enter_context(tc.tile_pool(name="psT", bufs=1, space="PSUM"))

            iota_i = const.tile([P, P], I32)
            nc.gpsimd.iota(iota_i[:], pattern=[[1, P]], base=0, channel_multiplier=0)
            piota_i = const.tile([P, 1], I32)
            nc.gpsimd.iota(piota_i[:], pattern=[[0, 1]], base=0, channel_multiplier=1)
            piota_f = const.tile([P, 1], F32)
            nc.vector.tensor_copy(out=piota_f[:], in_=piota_i[:])
            iota_f = const.tile([P, P], F32)
            nc.vector.tensor_copy(out=iota_f[:], in_=iota_i[:])
            identF = const.tile([P, P], F32)
            nc.vector.tensor_tensor(out=identF[:], in0=iota_f[:],
                                    in1=piota_f[:].to_broadcast([P, P]), op=OP.is_equal)
            if add_bias:
                bb = const.tile([P, D1], F32)
                nc.sync.dma_start(out=bb[:], in_=BB[:, :])
            w2e_sb = []
            for cb in range(D1 // P):
                t = const.tile([P, OUTC + 2], F16, tag=f"w2e_{cb}")
                nc.sync.dma_start(out=t[:], in_=W2E[cb * P:(cb + 1) * P, :])
                w2e_sb.append(t)

            def flush_b(w, po0, po1, pd):
                dr = fp.tile([P, H1], F32, tag="dr")
                nc.scalar.activation(out=dr[:], in_=pd[:], func=AF.Copy,
                                     bias=1e-16)
                drr = fp.tile([P, H1], F32, tag="drr")
                nc.vector.reciprocal(out=drr[:], in_=dr[:])
                o1 = fp.tile([P, D1], F32, tag="o1")
                nc.vector.tensor_tensor(
                    out=o1[:, 0:256].rearrange("p (h c) -> p h c", h=4),
                    in0=po0[:].rearrange("p (h c) -> p h c", h=4),
                    in1=drr[:, 0:4].to_broadcast([P, 4, C1]), op=OP.mult)
                nc.vector.tensor_tensor(
                    out=o1[:, 256:512].rearrange("p (h c) -> p h c", h=4),
                    in0=po1[:].rearrange("p (h c) -> p h c", h=4),
                    in1=drr[:, 4:8].to_broadcast([P, 4, C1]), op=OP.mult)
                if add_bias:
                    nc.vector.tensor_tensor(out=o1[:], in0=o1[:], in1=bb[:], op=OP.add)
                ee = fp.tile([P, D1], F32, tag="ee")
                nc.scalar.activation(out=ee[:], in_=o1[:], func=AF.Exp)
                nc.vector.tensor_scalar(out=ee[:], in0=ee[:], scalar1=1.0,
                                        scalar2=-1.0, op0=OP.min, op1=OP.add)
                h2 = fp.tile([P, D1], F32, tag="h2")
                nc.vector.tensor_tensor(out=h2[:], in0=o1[:], in1=ee[:], op=OP.max)
                ph2 = psH.tile([P, OUTC + 2], F32, tag="ph2")
                for cb in range(D1 // P):
                    pt = psT.tile([P, P], F32, tag="pt")
                    nc.tensor.transpose(out=pt[:], in_=h2[:, cb * P:(cb + 1) * P],
                                        identity=identF[:])
                    h2t = cp.tile([P, P], F16, tag="h2t")
                    nc.scalar.activation(out=h2t[:], in_=pt[:], func=AF.Copy)
                    nc.tensor.matmul(out=ph2[:], lhsT=h2t[:], rhs=w2e_sb[cb][:],
                                     start=cb == 0, stop=cb == D1 // P - 1)
                stage = st.tile([P, OUTC + 2], F16, tag="stage")
                nc.scalar.activation(out=stage[:], in_=ph2[:], func=AF.Copy)
                nc.sync.dma_start(out=tab2[w * P:(w + 1) * P, :], in_=stage[:])

            MB = K * 512              # meta region base: [asrc 8 | adst 8 | onehot 128]
            pending = []
            for w in range(NWC):
                G = gp.tile([P, K * RS1], F16, tag="G")
                nc.sync.dma_start(out=G[:], in_=EDG[w])
                Gm = G[:, MB:].rearrange("p (k t) -> p k t", t=144)
                S = sp.tile([P, K * H1], F32, tag="S")
                nc.vector.tensor_tensor(
                    out=S[:].rearrange("p (k h) -> p k h", h=H1),
                    in0=Gm[:, :, 0:8], in1=Gm[:, :, 8:16], op=OP.add)
                LR = sp.tile([P, K * H1], F32, tag="LR")
                nc.scalar.activation(out=LR[:], in_=S[:], func=AF.Prelu, alpha=NEG_SLOPE)
                EX = sp.tile([P, K * H1], F16, tag="EX")
                nc.scalar.activation(out=EX[:], in_=LR[:], func=AF.Exp)

                po0 = ps0.tile([P, 256], F32, tag="po0")
                po1 = ps1.tile([P, 256], F32, tag="po1")
                pd = psD.tile([P, H1], F32, tag="pd")
                for g, (k0, k1) in enumerate(KG):
                    L = k1 - k0
                    Mg = mp.tile([P, L * 512], F16, tag=f"M{g}")
                    nc.vector.tensor_tensor(
                        out=Mg[:].rearrange("p (j c) -> p j c", c=C1),
                        in0=G[:, k0 * 512:k1 * 512].rearrange("p (j c) -> p j c", c=C1),
                        in1=EX[:, k0 * H1:k1 * H1].to_broadcast([P, L * H1, C1]),
                        op=OP.mult)
                    for k in range(L):
                        gk = k0 + k
                        lhsT = G[:, MB + gk * 144 + 16:MB + (gk + 1) * 144]
                        nc.tensor.matmul(out=po0[:], lhsT=lhsT,
                                         rhs=Mg[:, k * 512:k * 512 + 256],
                                         start=gk == 0, stop=gk == K - 1)
                        nc.tensor.matmul(out=po1[:], lhsT=lhsT,
                                         rhs=Mg[:, k * 512 + 256:(k + 1) * 512],
                                         start=gk == 0, stop=gk == K - 1)
                        nc.tensor.matmul(out=pd[:], lhsT=lhsT,
                                         rhs=EX[:, gk * H1:(gk + 1) * H1],
                                         start=gk == 0, stop=gk == K - 1)

                pending.append((w, po0, po1, pd))
                if len(pending) > 1:
                    flush_b(*pending.pop(0))
            flush_b(*pending.pop(0))
    _split_excess_waits(nc)
    return nc


def _build_c(K, OUTC, add_bias):
    """Layer-2 edge phase: coefficient folded into the selector
    (CMPX = onehot * ex), raw message rows stream straight into the
    matmul; flush = divide, (+b2,) log_softmax."""
    nc = bass.Bass("TRN2", target_bir_lowering=False, debug=False, num_devices=NCORES)
    EDG = nc.dram_tensor("EDG", [NWC, P, K * RS2], F16, kind="ExternalInput")
    BB = nc.dram_tensor("BB", [P, OUTC], F32, kind="ExternalInput")
    out_t = nc.dram_tensor("out", [NWC * P, OUTC], F32, kind="ExternalOutput")
    KG = _kgroups(K, 4)
    with tile.TileContext(nc) as tc:
        with ExitStack() as ctx:
            const = ctx.enter_context(tc.tile_pool(name="const", bufs=1))
            gp = ctx.enter_context(tc.tile_pool(name="gp", bufs=4))
            cp = ctx.enter_context(tc.tile_pool(name="cp", bufs=3))
            sp = ctx.enter_context(tc.tile_pool(name="sp", bufs=3))
            fp = ctx.enter_context(tc.tile_pool(name="fp", bufs=2))
            ps0 = ctx.enter_context(tc.tile_pool(name="ps0", bufs=2, space="PSUM"))
            psW = ctx.enter_context(tc.tile_pool(name="psW", bufs=1, space="PSUM"))
            spin_ab = _spin_init(nc, const)
            _spin(nc, spin_ab, psW, SPIN)

            iota_i = const.tile([P, P], I32)
            nc.gpsimd.iota(iota_i[:], pattern=[[1, P]], base=0, channel_multiplier=0)
            iotag = const.tile([P, K * P], F16)
            for k in range(K):
                nc.gpsimd.tensor_copy(out=iotag[:, k * P:(k + 1) * P], in_=iota_i[:])
            if add_bias:
                bb = const.tile([P, OUTC], F32)
                nc.sync.dma_start(out=bb[:], in_=BB[:, :])

            def flush_c(w, po):
                dr = fp.tile([P, 1], F32, tag="dr")
                nc.scalar.activation(out=dr[:], in_=po[:, 256:257], func=AF.Copy,
                                     bias=1e-16)
                drr = fp.tile([P, 1], F32, tag="drr")
                nc.vector.reciprocal(out=drr[:], in_=dr[:])
                z = fp.tile([P, OUTC], F32, tag="z")
                nc.vector.tensor_scalar(out=z[:], in0=po[:, 0:256], scalar1=drr[:, :1],
                                        scalar2=None, op0=OP.mult)
                if add_bias:
                    nc.vector.tensor_tensor(out=z[:], in0=z[:], in1=bb[:], op=OP.add)
                ee = fp.tile([P, OUTC], F32, tag="ee")
                se = fp.tile([P, 1], F32, tag="se")
                nc.scalar.activation(out=ee[:], in_=z[:], func=AF.Exp, accum_out=se[:])
                lse = fp.tile([P, 1], F32, tag="lse")
                nc.scalar.activation(out=lse[:], in_=se[:], func=AF.Ln)
                nc.vector.tensor_scalar(out=z[:], in0=z[:], scalar1=lse[:, :1],
                                        scalar2=None, op0=OP.subtract)
                nc.sync.dma_start(out=out_t[w * P:(w + 1) * P, :], in_=z[:])

            pending = []
            for w in range(NWC):
                G = gp.tile([P, K * RS2], F16, tag="G")
                nc.sync.dma_start(out=G[:], in_=EDG[w])
                Gv = G[:].rearrange("p (k t) -> p k t", t=RS2)
                S = sp.tile([P, K], F32, tag="S")
                nc.vector.tensor_tensor(
                    out=S[:].rearrange("p (k o) -> p k o", o=1),
                    in0=Gv[:, :, 257:258], in1=Gv[:, :, 258:259], op=OP.add)
                LR = sp.tile([P, K], F32, tag="LR")
                nc.scalar.activation(out=LR[:], in_=S[:], func=AF.Prelu, alpha=NEG_SLOPE)
                EX = sp.tile([P, K], F16, tag="EX")
                nc.scalar.activation(out=EX[:], in_=LR[:], func=AF.Exp)
                EXv = EX[:].rearrange("p (k o) -> p k o", o=1)

                po = ps0.tile([P, 257], F32, tag="po")
                for g, (k0, k1) in enumerate(KG):
                    L = k1 - k0
                    _spin(nc, spin_ab, psW, 2)
                    CMPg = cp.tile([P, L * P], F16, tag=f"C{g}")
                    nc.vector.tensor_tensor(
                        out=CMPg[:].rearrange("p (k q) -> p k q", q=P),
                        in0=iotag[:, k0 * P:k1 * P].rearrange("p (k q) -> p k q", q=P),
                        in1=Gv[:, k0:k1, 259:260].rearrange("p k o -> p (k o)")
                            .to_broadcast([P, L, P]),
                        op=OP.is_equal)
                    CMXg = cp.tile([P, L * P], F16, tag=f"X{g}")
                    nc.vector.tensor_tensor(
                        out=CMXg[:].rearrange("p (k q) -> p k q", q=P),
                        in0=CMPg[:].rearrange("p (k q) -> p k q", q=P),
                        in1=EXv[:, k0:k1, :].to_broadcast([P, L, P]), op=OP.mult)
                    for k in range(L):
                        gk = k0 + k
                        nc.tensor.matmul(out=po[:], lhsT=CMXg[:, k * P:(k + 1) * P],
                                         rhs=G[:, gk * RS2:gk * RS2 + 257],
                                         start=gk == 0, stop=gk == K - 1)

                pending.append((w, po))
                if len(pending) > 1:
                    flush_c(*pending.pop(0))
            flush_c(*pending.pop(0))
    _split_excess_waits(nc)
    return nc


def kernel(x, edge_index, W1, att_src1, att_dst1, b1, W2, att_src2, att_dst2, b2):
    x = np.asarray(x, np.float32)
    edge_index = np.asarray(edge_index)
    W1d = np.asarray(W1, np.float64)
    W2d = np.asarray(W2, np.float64)
    as1 = np.asarray(att_src1, np.float64)
    ad1 = np.asarray(att_dst1, np.float64)
    as2 = np.asarray(att_src2, np.float64)
    ad2 = np.asarray(att_dst2, np.float64)
    b1 = np.asarray(b1, np.float32)
    b2 = np.asarray(b2, np.float32)
    N, D1 = x.shape
    H1, C1 = att_src1.shape
    OUTC = W2.shape[1]
    NW = NCORES * NWC
    NPC = NWC * P
    core_ids = list(range(NCORES))
    npc_in = N // NCORES

    src = np.concatenate([edge_index[0], np.arange(N)]).astype(np.int64)
    dst = np.concatenate([edge_index[1], np.arange(N)]).astype(np.int64)
    win_of, slot_of, K = _pack_windows(dst, N, NW)

    # edge -> (window, chunk, partition) in window-major stable order
    w_e = win_of[dst]
    eorder = np.argsort(w_e, kind="stable")
    sw = w_e[eorder]
    counts = np.bincount(sw, minlength=NW)
    starts = np.concatenate([[0], np.cumsum(counts)[:-1]])
    pos = np.arange(len(sw)) - starts[sw]
    k_e = (pos // P).astype(np.int64)
    p_e = (pos % P).astype(np.int64)
    s_e = src[eorder]
    d_e = dst[eorder]
    row_of_node = win_of.astype(np.int64) * P + slot_of  # global table row

    # ---- Launch A: per-node table [h_lo, asrc, h_hi, adst] ----
    asd_s = np.zeros((D1, H1))
    asd_d = np.zeros((D1, H1))
    for h in range(H1):
        asd_s[h * C1:(h + 1) * C1, h] = as1[h]
        asd_d[h * C1:(h + 1) * C1, h] = ad1[h]
    W1E = np.concatenate([W1d[:, 0:256], W1d @ asd_s, W1d[:, 256:512], W1d @ asd_d],
                         axis=1).astype(np.float16)
    nc_a = _build_a(D1, NPC)
    in_maps = []
    for c in range(NCORES):
        xo = np.zeros((NPC, D1), np.float16)
        xo[:npc_in] = x[c * npc_in:(c + 1) * npc_in].astype(np.float16)
        in_maps.append({"xT": np.ascontiguousarray(xo.T), "W1E": W1E})
    res_a = run_bass_kernel_spmd(nc_a, in_maps, core_ids)
    tab1 = np.concatenate([res_a.results[c]["tab1"][:npc_in] for c in range(NCORES)], axis=0)
    h_lo = tab1[:, 0:256]
    a_src_n = tab1[:, 256:264]
    h_hi = tab1[:, 264:520]
    a_dst_n = tab1[:, 520:528]

    # ---- Launch B: layer-1 edge phase ----
    W2E = np.concatenate([W2d, W2d @ as2.T, W2d @ ad2.T], axis=1).astype(np.float16)
    BB1 = np.tile(b1.reshape(1, D1), (P, 1))
    nc_b = _build_b(K, D1, H1, OUTC, bool(np.any(b1)))
    in_maps = []
    for c in range(NCORES):
        m = (sw >= c * NWC) & (sw < (c + 1) * NWC)
        lw, kk, pp = sw[m] - c * NWC, k_e[m], p_e[m]
        sm, dm = s_e[m], d_e[m]
        EDGm = np.zeros((NWC, P, K, 512), np.float16)
        EDGr = np.zeros((NWC, P, K, 144), np.float16)
        EDGm[lw, pp, kk, 0:256] = h_lo[sm]
        EDGm[lw, pp, kk, 256:512] = h_hi[sm]
        EDGr[lw, pp, kk, 0:8] = a_src_n[sm]
        EDGr[lw, pp, kk, 8:16] = a_dst_n[dm]
        EDGr[lw, pp, kk, 16 + slot_of[dm]] = 1.0
        EDG = np.ascontiguousarray(np.concatenate(
            [EDGm.reshape(NWC, P, K * 512), EDGr.reshape(NWC, P, K * 144)], axis=2))
        in_maps.append({"EDG": EDG, "W2E": W2E, "BB": BB1})
    res_b = run_bass_kernel_spmd(nc_b, in_maps, core_ids)
    tab2 = np.concatenate([res_b.results[c]["tab2"] for c in range(NCORES)], axis=0)
    h2p = tab2[:, 0:256]
    a_src2_n = tab2[:, 256]
    a_dst2_n = tab2[:, 257]

    # ---- Launch C: layer-2 edge phase + log_softmax ----
    BB2 = np.tile(b2.reshape(1, OUTC), (P, 1))
    nc_c = _build_c(K, OUTC, bool(np.any(b2)))
    in_maps = []
    sr = row_of_node[s_e]
    dr_ = row_of_node[d_e]
    for c in range(NCORES):
        m = (sw >= c * NWC) & (sw < (c + 1) * NWC)
        lw, kk, pp = sw[m] - c * NWC, k_e[m], p_e[m]
        srm, drm = sr[m], dr_[m]
        EDG = np.zeros((NWC, P, K, RS2), np.float16)
        EDG[:, :, :, 259] = 255.0
        EDG[lw, pp, kk, 0:256] = h2p[srm]
        EDG[lw, pp, kk, 256] = 1.0
        EDG[lw, pp, kk, 257] = a_src2_n[srm]
        EDG[lw, pp, kk, 258] = a_dst2_n[drm]
        EDG[lw, pp, kk, 259] = slot_of[d_e[m]].astype(np.float16)
        in_maps.append({"EDG": EDG.reshape(NWC, P, K * RS2), "BB": BB2})
    res_c = run_bass_kernel_spmd(nc_c, in_maps, core_ids)
    rows = np.concatenate([res_c.results[c]["out"] for c in range(NCORES)], axis=0)
    return np.ascontiguousarray(rows[row_of_node]).astype(np.float32)


# revision 21
# speedup vs baseline: 1.0515x; 1.0061x over previous
"""2-layer GAT (GATConv x2 + log_softmax) on 8 Trainium2 NeuronCores.

fp16 streaming design (SPMD across 8 cores; host does index marshaling
between launches, device does all arithmetic):
  - Host bin-packs the 20000 nodes into 160 windows of 128 dst slots each
    (20 windows/core), balancing in-degree so every window holds ~2125
    edges -> K = ceil(max/128) = 17 chunks of 128 edges.
  - Launch A: h = x@W1 with the attention alphas fused in via host-extended
    weights [W1 | W1@asd_src | W1@asd_dst]; emits a per-node fp16 table.
  - Host pre-gathers per-edge rows (halo exchange): for each edge slot a
    656-col fp16 record [h_lo(256) | h_hi(256) | asrc(8) | adst(8) |
    one-hot dst selector(128)].
  - Launch B: per window one bulk DMA; ACT computes exp(leakyrelu(alpha)),
    DVE scales messages by the per-head coefficients (sliced per k-group so
    selector matmuls start early); PE scatter-adds messages+denominators
    into PSUM via the shipped selectors; flush: divide, ELU, @[W2 | W2@a2]
    -> layer-2 fp16 table rows.
  - Launch C: layer 2 (H=1) folds the coefficient into the selector
    (CMPX = onehot(dl) * ex), so the 256-wide messages stream raw into the
    matmul; flush = log_softmax.
All matmul operands fp16 (1 cyc/row, f32 PSUM accumulation). All big
elementwise work stays on DVE: GpSimd shares SBUF ports with it, so
running both concurrently slows DVE ~2.7x (measured) and nets nothing.
"""
import heapq
import numpy as np
from contextlib import ExitStack

import concourse.bass as bass
import concourse.tile as tile
from concourse import mybir
from concourse.bass_utils import run_bass_kernel_spmd

F16 = mybir.dt.float16
F32 = mybir.dt.float32
I32 = mybir.dt.int32
AF = mybir.ActivationFunctionType
OP = mybir.AluOpType
P = 128
NCORES = 8
NWC = 20                 # windows per core
NEG_SLOPE = 0.2
RS1 = 656                # layer-1 record: 256+256+8+8 + one-hot(128)
MS1 = 520                # layer-1 message block: 256 + 8(ex) + 256
RS2 = 260                # layer-2 record: 256 + 1.0 + asrc + adst + dl
SPIN = 24                # PE warm-up matmuls (HAM releases the clock gate
                         # only after ~3.4us of sustained PE activity)


def _split_excess_waits(nc, max_waits=1):
    """This walrus build rejects instructions with >~2 sync waits; move excess
    waits onto same-engine wait-only instructions placed just before."""
    cnt = 0
    for f in nc.m.functions:
        for bb in f.blocks:
            new_insts = []
            for inst in bb.instructions:
                si = inst.sync_info
                if si is not None and si.on_wait and len(si.on_wait) > max_waits:
                    waits = list(si.on_wait)
                    extra, keep = waits[:-max_waits], waits[-max_waits:]
                    for w in extra:
                        cnt += 1
                        nop = mybir.InstNoOp(name=f"wsplit-{cnt}-{inst.name}", ins=[], outs=[])
                        nop.engine = inst.engine
                        nop.sync_info = mybir.SyncInfo(on_wait=[w], on_update=[])
                        new_insts.append(nop)
                    si.on_wait = keep
                new_insts.append(inst)
            bb.instructions = new_insts
    return cnt


def _pack_windows(dst, N, nw):
    """Greedy balance in-degree over nw windows of P slots. Returns
    win_of[N], slot_of[N], K (edge chunks per window)."""
    deg = np.bincount(dst, minlength=N)
    order = np.argsort(-deg, kind="stable")
    wload = np.zeros(nw, np.int64)
    wcnt = np.zeros(nw, np.int64)
    win_of = np.zeros(N, np.int32)
    slot_of = np.zeros(N, np.int32)
    heap = [(0, w) for w in range(nw)]
    heapq.heapify(heap)
    for n in order:
        while True:
            load, w = heapq.heappop(heap)
            if wcnt[w] < P:
                break
        win_of[n] = w
        slot_of[n] = wcnt[w]
        wcnt[w] += 1
        wload[w] += deg[n]
        if wcnt[w] < P:
            heapq.heappush(heap, (wload[w], w))
    K = int(np.ceil(wload.max() / P))
    return win_of, slot_of, K


def _kgroups(K, ng):
    """Split range(K) into ng contiguous groups for DVE/PE pipelining."""
    out = []
    base = 0
    for g in range(ng):
        n = (K - base + (ng - g) - 1) // (ng - g)
        out.append((base, base + n))
        base += n
    return out


def _spin_init(nc, const):
    a = const.tile([P, P], F16, tag="spin_a")
    nc.vector.memset(a[:], 1.0)
    b = const.tile([P, 512], F16, tag="spin_b")
    nc.vector.memset(b[:], 0.5)
    return a, b


def _spin(nc, spin_ab, pool, n):
    """Dependency-free matmuls that keep the PE HAM activity monitor busy
    (the clock gate drops to 1.2 GHz after ~3.4us of low PE activity);
    issued ahead of real matmuls so they fill operand-wait gaps."""
    a, b = spin_ab
    ps = pool.tile([P, 512], F32, tag="spin_ps")
    for i in range(n):
        nc.tensor.matmul(out=ps[:], lhsT=a[:], rhs=b[:], start=i == 0,
                         stop=i == n - 1)


def _build_a(D1, NPC):
    """h = x@W1 + fused alphas. out row = [po0(264) | po1(264)] =
    [h[0:256], asrc(8), h[256:512], adst(8)]."""
    nc = bass.Bass("TRN2", target_bir_lowering=False, debug=False, num_devices=NCORES)
    xT = nc.dram_tensor("xT", [D1, NPC], F16, kind="ExternalInput")
    W1E = nc.dram_tensor("W1E", [D1, 2 * 264], F16, kind="ExternalInput")
    tab1 = nc.dram_tensor("tab1", [NPC, 528], F16, kind="ExternalOutput")
    KB = D1 // P
    with tile.TileContext(nc) as tc:
        with ExitStack() as ctx:
            const = ctx.enter_context(tc.tile_pool(name="const", bufs=1))
            work = ctx.enter_context(tc.tile_pool(name="work", bufs=3))
            ps0 = ctx.enter_context(tc.tile_pool(name="ps0", bufs=2, space="PSUM"))
            ps1 = ctx.enter_context(tc.tile_pool(name="ps1", bufs=2, space="PSUM"))
            psW = ctx.enter_context(tc.tile_pool(name="psW", bufs=1, space="PSUM"))
            spin_ab = _spin_init(nc, const)
            _spin(nc, spin_ab, psW, SPIN)
            xsb, w0sb, w1sb = [], [], []
            for kb in range(KB):
                t = const.tile([P, NPC], F16, tag=f"x_{kb}")
                nc.sync.dma_start(out=t[:], in_=xT[kb * P:(kb + 1) * P, :])
                xsb.append(t)
                t0 = const.tile([P, 264], F16, tag=f"w0_{kb}")
                nc.sync.dma_start(out=t0[:], in_=W1E[kb * P:(kb + 1) * P, 0:264])
                w0sb.append(t0)
                t1 = const.tile([P, 264], F16, tag=f"w1_{kb}")
                nc.sync.dma_start(out=t1[:], in_=W1E[kb * P:(kb + 1) * P, 264:528])
                w1sb.append(t1)
            for t_i in range(NPC // P):
                po0 = ps0.tile([P, 264], F32, tag="po0")
                po1 = ps1.tile([P, 264], F32, tag="po1")
                for kb in range(KB):
                    lhsT = xsb[kb][:, t_i * P:(t_i + 1) * P]
                    nc.tensor.matmul(out=po0[:], lhsT=lhsT, rhs=w0sb[kb][:],
                                     start=kb == 0, stop=kb == KB - 1)
                for kb in range(KB):
                    lhsT = xsb[kb][:, t_i * P:(t_i + 1) * P]
                    nc.tensor.matmul(out=po1[:], lhsT=lhsT, rhs=w1sb[kb][:],
                                     start=kb == 0, stop=kb == KB - 1)
                stage = work.tile([P, 528], F16, tag="stage")
                nc.scalar.activation(out=stage[:, 0:264], in_=po0[:], func=AF.Copy)
                nc.vector.tensor_copy(out=stage[:, 264:528], in_=po1[:])
                nc.sync.dma_start(out=tab1[t_i * P:(t_i + 1) * P, :], in_=stage[:])
    _split_excess_waits(nc)
    return nc


def _build_b(K, D1, H1, OUTC, add_bias):
    """Layer-1 edge phase + flush into the layer-2 table.

    Elementwise work split: DVE keeps msg_lo scaling + part of the selector
    compares + the (PSUM-bound) flush; GpSimd takes msg_hi scaling, the rest
    of the compares, and the small copies. Both sliced into 4 k-groups so
    selector matmuls start early and the PE never idles long enough for the
    HAM clock gate to drop."""
    C1 = D1 // H1
    nc = bass.Bass("TRN2", target_bir_lowering=False, debug=False, num_devices=NCORES)
    EDG = nc.dram_tensor("EDG", [NWC, P, K * RS1], F16, kind="ExternalInput")
    W2E = nc.dram_tensor("W2E", [D1, OUTC + 2], F16, kind="ExternalInput")
    BB = nc.dram_tensor("BB", [P, D1], F32, kind="ExternalInput")
    tab2 = nc.dram_tensor("tab2", [NWC * P, OUTC + 2], F16, kind="ExternalOutput")
    KG = _kgroups(K, 4)
    with tile.TileContext(nc) as tc:
        with ExitStack() as ctx:
            const = ctx.enter_context(tc.tile_pool(name="const", bufs=1))
            gp = ctx.enter_context(tc.tile_pool(name="gp", bufs=4))
            mp = ctx.enter_context(tc.tile_pool(name="mp", bufs=3))
            cp = ctx.enter_context(tc.tile_pool(name="cp", bufs=2))
            sp = ctx.enter_context(tc.tile_pool(name="sp", bufs=3))
            fp = ctx.enter_context(tc.tile_pool(name="fp", bufs=2))
            st = ctx.enter_context(tc.tile_pool(name="st", bufs=2))
            ps0 = ctx.enter_context(tc.tile_pool(name="ps0", bufs=2, space="PSUM"))
            ps1 = ctx.enter_context(tc.tile_pool(name="ps1", bufs=2, space="PSUM"))
            psD = ctx.enter_context(tc.tile_pool(name="psD", bufs=2, space="PSUM"))
            psH = ctx.enter_context(tc.tile_pool(name="psH", bufs=1, space="PSUM"))
            psT = ctx.enter_context(tc.tile_pool(name="psT", bufs=1, space="PSUM"))

            iota_i = const.tile([P, P], I32)
            nc.gpsimd.iota(iota_i[:], pattern=[[1, P]], base=0, channel_multiplier=0)
            piota_i = const.tile([P, 1], I32)
            nc.gpsimd.iota(piota_i[:], pattern=[[0, 1]], base=0, channel_multiplier=1)
            piota_f = const.tile([P, 1], F32)
            nc.vector.tensor_copy(out=piota_f[:], in_=piota_i[:])
            iota_f = const.tile([P, P], F32)
            nc.vector.tensor_copy(out=iota_f[:], in_=iota_i[:])
            identF = const.tile([P, P], F32)
            nc.vector.tensor_tensor(out=identF[:], in0=iota_f[:],
                                    in1=piota_f[:].to_broadcast([P, P]), op=OP.is_equal)
            if add_bias:
                bb = const.tile([P, D1], F32)
                nc.sync.dma_start(out=bb[:], in_=BB[:, :])
            w2e_sb = []
            for cb in range(D1 // P):
                t = const.tile([P, OUTC + 2], F16, tag=f"w2e_{cb}")
                nc.sync.dma_start(out=t[:], in_=W2E[cb * P:(cb + 1) * P, :])
                w2e_sb.append(t)

            def flush_b(w, po0, po1, pd):
                dr = fp.tile([P, H1], F32, tag="dr")
                nc.scalar.activation(out=dr[:], in_=pd[:], func=AF.Copy,
                                     bias=1e-16)
                drr = fp.tile([P, H1], F32, tag="drr")
                nc.vector.reciprocal(out=drr[:], in_=dr[:])
                o1 = fp.tile([P, D1], F32, tag="o1")
                for h in range(H1):
                    src_po = po0 if h < 4 else po1
                    nc.scalar.activation(
                        out=o1[:, h * C1:(h + 1) * C1],
                        in_=src_po[:, (h % 4) * C1:(h % 4 + 1) * C1],
                        func=AF.Copy, scale=drr[:, h:h + 1])
                if add_bias:
                    nc.vector.tensor_tensor(out=o1[:], in0=o1[:], in1=bb[:], op=OP.add)
                ee = fp.tile([P, D1], F32, tag="ee")
                nc.scalar.activation(out=ee[:], in_=o1[:], func=AF.Exp)
                nc.vector.tensor_scalar(out=ee[:], in0=ee[:], scalar1=1.0,
                                        scalar2=-1.0, op0=OP.min, op1=OP.add)
                h2 = fp.tile([P, D1], F32, tag="h2")
                nc.vector.tensor_tensor(out=h2[:], in0=o1[:], in1=ee[:], op=OP.max)
                ph2 = psH.tile([P, OUTC + 2], F32, tag="ph2")
                for cb in range(D1 // P):
                    pt = psT.tile([P, P], F32, tag="pt")
                    nc.tensor.transpose(out=pt[:], in_=h2[:, cb * P:(cb + 1) * P],
                                        identity=identF[:])
                    h2t = cp.tile([P, P], F16, tag="h2t")
                    nc.scalar.activation(out=h2t[:], in_=pt[:], func=AF.Copy)
                    nc.tensor.matmul(out=ph2[:], lhsT=h2t[:], rhs=w2e_sb[cb][:],
                                     start=cb == 0, stop=cb == D1 // P - 1)
                stage = st.tile([P, OUTC + 2], F16, tag="stage")
                nc.scalar.activation(out=stage[:], in_=ph2[:], func=AF.Copy)
                nc.sync.dma_start(out=tab2[w * P:(w + 1) * P, :], in_=stage[:])

            MB = K * 512              # meta region base: [asrc 8 | adst 8 | onehot 128]
            pending = []
            for w in range(NWC):
                G = gp.tile([P, K * RS1], F16, tag="G")
                nc.sync.dma_start(out=G[:], in_=EDG[w])
                Gm = G[:, MB:].rearrange("p (k t) -> p k t", t=144)
                S = sp.tile([P, K * H1], F32, tag="S")
                nc.vector.tensor_tensor(
                    out=S[:].rearrange("p (k h) -> p k h", h=H1),
                    in0=Gm[:, :, 0:8], in1=Gm[:, :, 8:16], op=OP.add)
                LR = sp.tile([P, K * H1], F32, tag="LR")
                nc.scalar.activation(out=LR[:], in_=S[:], func=AF.Prelu, alpha=NEG_SLOPE)
                EX = sp.tile([P, K * H1], F16, tag="EX")
                nc.scalar.activation(out=EX[:], in_=LR[:], func=AF.Exp)

                po0 = ps0.tile([P, 256], F32, tag="po0")
                po1 = ps1.tile([P, 256], F32, tag="po1")
                pd = psD.tile([P, H1], F32, tag="pd")
                for g, (k0, k1) in enumerate(KG):
                    L = k1 - k0
                    Mg = mp.tile([P, L * 512], F16, tag=f"M{g}")
                    nc.vector.tensor_tensor(
                        out=Mg[:].rearrange("p (j c) -> p j c", c=C1),
                        in0=G[:, k0 * 512:k1 * 512].rearrange("p (j c) -> p j c", c=C1),
                        in1=EX[:, k0 * H1:k1 * H1].to_broadcast([P, L * H1, C1]),
                        op=OP.mult)
                    for k in range(L):
                        gk = k0 + k
                        lhsT = G[:, MB + gk * 144 + 16:MB + (gk + 1) * 144]
                        nc.tensor.matmul(out=po0[:], lhsT=lhsT,
                                         rhs=Mg[:, k * 512:k * 512 + 256],
                                         start=gk == 0, stop=gk == K - 1)
                        nc.tensor.matmul(out=po1[:], lhsT=lhsT,
                                         rhs=Mg[:, k * 512 + 256:(k + 1) * 512],
                                         start=gk == 0, stop=gk == K - 1)
                        nc.tensor.matmul(out=pd[:], lhsT=lhsT,
                                         rhs=EX[:, gk * H1:(gk + 1) * H1],
                                         start=gk == 0, stop=gk == K - 1)

                pending.append((w, po0, po1, pd))
                if len(pending) > 1:
                    flush_b(*pending.pop(0))
            flush_b(*pending.pop(0))
    _split_excess_waits(nc)
    return nc


def _build_c(K, OUTC, add_bias):
    """Layer-2 edge phase: coefficient folded into the selector
    (CMPX = onehot * ex), raw message rows stream straight into the
    matmul; flush = divide, (+b2,) log_softmax."""
    nc = bass.Bass("TRN2", target_bir_lowering=False, debug=False, num_devices=NCORES)
    EDG = nc.dram_tensor("EDG", [NWC, P, K * RS2], F16, kind="ExternalInput")
    BB = nc.dram_tensor("BB", [P, OUTC], F32, kind="ExternalInput")
    out_t = nc.dram_tensor("out", [NWC * P, OUTC], F32, kind="ExternalOutput")
    KG = _kgroups(K, 4)
    with tile.TileContext(nc) as tc:
        with ExitStack() as ctx:
            const = ctx.enter_context(tc.tile_pool(name="const", bufs=1))
            gp = ctx.enter_context(tc.tile_pool(name="gp", bufs=4))
            cp = ctx.enter_context(tc.tile_pool(name="cp", bufs=3))
            sp = ctx.enter_context(tc.tile_pool(name="sp", bufs=3))
            fp = ctx.enter_context(tc.tile_pool(name="fp", bufs=2))
            ps0 = ctx.enter_context(tc.tile_pool(name="ps0", bufs=2, space="PSUM"))
            psW = ctx.enter_context(tc.tile_pool(name="psW", bufs=1, space="PSUM"))
            spin_ab = _spin_init(nc, const)
            _spin(nc, spin_ab, psW, SPIN)

            iota_i = const.tile([P, P], I32)
            nc.gpsimd.iota(iota_i[:], pattern=[[1, P]], base=0, channel_multiplier=0)
            iotag = const.tile([P, K * P], F16)
            for k in range(K):
                nc.gpsimd.tensor_copy(out=iotag[:, k * P:(k + 1) * P], in_=iota_i[:])
            if add_bias:
                bb = const.tile([P, OUTC], F32)
                nc.sync.dma_start(out=bb[:], in_=BB[:, :])

            def flush_c(w, po):
                dr = fp.tile([P, 1], F32, tag="dr")
                nc.scalar.activation(out=dr[:], in_=po[:, 256:257], func=AF.Copy,
                                     bias=1e-16)
                drr = fp.tile([P, 1], F32, tag="drr")
                nc.vector.reciprocal(out=drr[:], in_=dr[:])
                z = fp.tile([P, OUTC], F32, tag="z")
                nc.scalar.activation(out=z[:], in_=po[:, 0:256], func=AF.Copy,
                                     scale=drr[:, :1])
                if add_bias:
                    nc.vector.tensor_tensor(out=z[:], in0=z[:], in1=bb[:], op=OP.add)
                ee = fp.tile([P, OUTC], F32, tag="ee")
                se = fp.tile([P, 1], F32, tag="se")
                nc.scalar.activation(out=ee[:], in_=z[:], func=AF.Exp, accum_out=se[:])
                lse = fp.tile([P, 1], F32, tag="lse")
                nc.scalar.activation(out=lse[:], in_=se[:], func=AF.Ln)
                nc.vector.tensor_scalar(out=z[:], in0=z[:], scalar1=lse[:, :1],
                                        scalar2=None, op0=OP.subtract)
                nc.sync.dma_start(out=out_t[w * P:(w + 1) * P, :], in_=z[:])

            pending = []
            for w in range(NWC):
                G = gp.tile([P, K * RS2], F16, tag="G")
                nc.sync.dma_start(out=G[:], in_=EDG[w])
                Gv = G[:].rearrange("p (k t) -> p k t", t=RS2)
                S = sp.tile([P, K], F32, tag="S")
                nc.vector.tensor_tensor(
                    out=S[:].rearrange("p (k o) -> p k o", o=1),
                    in0=Gv[:, :, 257:258], in1=Gv[:, :, 258:259], op=OP.add)
                LR = sp.tile([P, K], F32, tag="LR")
                nc.scalar.activation(out=LR[:], in_=S[:], func=AF.Prelu, alpha=NEG_SLOPE)
                EX = sp.tile([P, K], F16, tag="EX")
                nc.scalar.activation(out=EX[:], in_=LR[:], func=AF.Exp)
                EXv = EX[:].rearrange("p (k o) -> p k o", o=1)

                po = ps0.tile([P, 257], F32, tag="po")
                for g, (k0, k1) in enumerate(KG):
                    L = k1 - k0
                    _spin(nc, spin_ab, psW, 2)
                    CMPg = cp.tile([P, L * P], F16, tag=f"C{g}")
                    nc.vector.tensor_tensor(
                        out=CMPg[:].rearrange("p (k q) -> p k q", q=P),
                        in0=iotag[:, k0 * P:k1 * P].rearrange("p (k q) -> p k q", q=P),
                        in1=Gv[:, k0:k1, 259:260].rearrange("p k o -> p (k o)")
                            .to_broadcast([P, L, P]),
                        op=OP.is_equal)
                    CMXg = cp.tile([P, L * P], F16, tag=f"X{g}")
                    nc.vector.tensor_tensor(
                        out=CMXg[:].rearrange("p (k q) -> p k q", q=P),
                        in0=CMPg[:].rearrange("p (k q) -> p k q", q=P),
                        in1=EXv[:, k0:k1, :].to_broadcast([P, L, P]), op=OP.mult)
                    for k in range(L):
                        gk = k0 + k
                        nc.tensor.matmul(out=po[:], lhsT=CMXg[:, k * P:(k + 1) * P],
                                         rhs=G[:, gk * RS2:gk * RS2 + 257],
                                         start=gk == 0, stop=gk == K - 1)

                pending.append((w, po))
                if len(pending) > 1:
                    flush_c(*pending.pop(0))
            flush_c(*pending.pop(0))
    _split_excess_waits(nc)
    return nc


def kernel(x, edge_index, W1, att_src1, att_dst1, b1, W2, att_src2, att_dst2, b2):
    x = np.asarray(x, np.float32)
    edge_index = np.asarray(edge_index)
    W1d = np.asarray(W1, np.float64)
    W2d = np.asarray(W2, np.float64)
    as1 = np.asarray(att_src1, np.float64)
    ad1 = np.asarray(att_dst1, np.float64)
    as2 = np.asarray(att_src2, np.float64)
    ad2 = np.asarray(att_dst2, np.float64)
    b1 = np.asarray(b1, np.float32)
    b2 = np.asarray(b2, np.float32)
    N, D1 = x.shape
    H1, C1 = att_src1.shape
    OUTC = W2.shape[1]
    NW = NCORES * NWC
    NPC = NWC * P
    core_ids = list(range(NCORES))
    npc_in = N // NCORES

    src = np.concatenate([edge_index[0], np.arange(N)]).astype(np.int64)
    dst = np.concatenate([edge_index[1], np.arange(N)]).astype(np.int64)
    win_of, slot_of, K = _pack_windows(dst, N, NW)

    # edge -> (window, chunk, partition) in window-major stable order
    w_e = win_of[dst]
    eorder = np.argsort(w_e, kind="stable")
    sw = w_e[eorder]
    counts = np.bincount(sw, minlength=NW)
    starts = np.concatenate([[0], np.cumsum(counts)[:-1]])
    pos = np.arange(len(sw)) - starts[sw]
    k_e = (pos // P).astype(np.int64)
    p_e = (pos % P).astype(np.int64)
    s_e = src[eorder]
    d_e = dst[eorder]
    row_of_node = win_of.astype(np.int64) * P + slot_of  # global table row

    # ---- Launch A: per-node table [h_lo, asrc, h_hi, adst] ----
    asd_s = np.zeros((D1, H1))
    asd_d = np.zeros((D1, H1))
    for h in range(H1):
        asd_s[h * C1:(h + 1) * C1, h] = as1[h]
        asd_d[h * C1:(h + 1) * C1, h] = ad1[h]
    W1E = np.concatenate([W1d[:, 0:256], W1d @ asd_s, W1d[:, 256:512], W1d @ asd_d],
                         axis=1).astype(np.float16)
    nc_a = _build_a(D1, NPC)
    in_maps = []
    for c in range(NCORES):
        xo = np.zeros((NPC, D1), np.float16)
        xo[:npc_in] = x[c * npc_in:(c + 1) * npc_in].astype(np.float16)
        in_maps.append({"xT": np.ascontiguousarray(xo.T), "W1E": W1E})
    res_a = run_bass_kernel_spmd(nc_a, in_maps, core_ids)
    tab1 = np.concatenate([res_a.results[c]["tab1"][:npc_in] for c in range(NCORES)], axis=0)
    h_lo = tab1[:, 0:256]
    a_src_n = tab1[:, 256:264]
    h_hi = tab1[:, 264:520]
    a_dst_n = tab1[:, 520:528]

    # ---- Launch B: layer-1 edge phase ----
    W2E = np.concatenate([W2d, W2d @ as2.T, W2d @ ad2.T], axis=1).astype(np.float16)
    BB1 = np.tile(b1.reshape(1, D1), (P, 1))
    nc_b = _build_b(K, D1, H1, OUTC, bool(np.any(b1)))
    in_maps = []
    for c in range(NCORES):
        m = (sw >= c * NWC) & (sw < (c + 1) * NWC)
        lw, kk, pp = sw[m] - c * NWC, k_e[m], p_e[m]
        sm, dm = s_e[m], d_e[m]
        EDGm = np.zeros((NWC, P, K, 512), np.float16)
        EDGr = np.zeros((NWC, P, K, 144), np.float16)
        EDGm[lw, pp, kk, 0:256] = h_lo[sm]
        EDGm[lw, pp, kk, 256:512] = h_hi[sm]
        EDGr[lw, pp, kk, 0:8] = a_src_n[sm]
        EDGr[lw, pp, kk, 8:16] = a_dst_n[dm]
        EDGr[lw, pp, kk, 16 + slot_of[dm]] = 1.0
        EDG = np.ascontiguousarray(np.concatenate(
            [EDGm.reshape(NWC, P, K * 512), EDGr.reshape(NWC, P, K * 144)], axis=2))
        in_maps.append({"EDG": EDG, "W2E": W2E, "BB": BB1})
    res_b = run_bass_kernel_spmd(nc_b, in_maps, core_ids)
    tab2 = np.concatenate([res_b.results[c]["tab2"] for c in range(NCORES)], axis=0)
    h2p = tab2[:, 0:256]
    a_src2_n = tab2[:, 256]
    a_dst2_n = tab2[:, 257]

    # ---- Launch C: layer-2 edge phase + log_softmax ----
    BB2 = np.tile(b2.reshape(1, OUTC), (P, 1))
    nc_c = _build_c(K, OUTC, bool(np.any(b2)))
    in_maps = []
    sr = row_of_node[s_e]
    dr_ = row_of_node[d_e]
    for c in range(NCORES):
        m = (sw >= c * NWC) & (sw < (c + 1) * NWC)
        lw, kk, pp = sw[m] - c * NWC, k_e[m], p_e[m]
        srm, drm = sr[m], dr_[m]
        EDG = np.zeros((NWC, P, K, RS2), np.float16)
        EDG[:, :, :, 259] = 255.0
        EDG[lw, pp, kk, 0:256] = h2p[srm]
        EDG[lw, pp, kk, 256] = 1.0
        EDG[lw, pp, kk, 257] = a_src2_n[srm]
        EDG[lw, pp, kk, 258] = a_dst2_n[drm]
        EDG[lw, pp, kk, 259] = slot_of[d_e[m]].astype(np.float16)
        in_maps.append({"EDG": EDG.reshape(NWC, P, K * RS2), "BB": BB2})
    res_c = run_bass_kernel_spmd(nc_c, in_maps, core_ids)
    rows = np.concatenate([res_c.results[c]["out"] for c in range(NCORES)], axis=0)
    return np.ascontiguousarray(rows[row_of_node]).astype(np.float32)


# revision 22
# speedup vs baseline: 1.0532x; 1.0016x over previous
"""2-layer GAT (GATConv x2 + log_softmax) on 8 Trainium2 NeuronCores.

fp16 streaming design (SPMD across 8 cores; host does index marshaling
between launches, device does all arithmetic):
  - Host bin-packs the 20000 nodes into 160 windows of 128 dst slots each
    (20 windows/core), balancing in-degree so every window holds ~2125
    edges -> K = ceil(max/128) = 17 chunks of 128 edges.
  - Launch A: h = x@W1 with the attention alphas fused in via host-extended
    weights [W1 | W1@asd_src | W1@asd_dst]; emits a per-node fp16 table.
  - Host pre-gathers per-edge rows (halo exchange): for each edge slot a
    656-col fp16 record [h_lo(256) | h_hi(256) | asrc(8) | adst(8) |
    one-hot dst selector(128)].
  - Launch B: per window one bulk DMA; ACT computes exp(leakyrelu(alpha)),
    DVE scales messages by the per-head coefficients (sliced per k-group so
    selector matmuls start early); PE scatter-adds messages+denominators
    into PSUM via the shipped selectors; flush: divide, ELU, @[W2 | W2@a2]
    -> layer-2 fp16 table rows.
  - Launch C: layer 2 (H=1) folds the coefficient into the selector
    (CMPX = onehot(dl) * ex), so the 256-wide messages stream raw into the
    matmul; flush = log_softmax.
All matmul operands fp16 (1 cyc/row, f32 PSUM accumulation). All big
elementwise work stays on DVE: GpSimd shares SBUF ports with it, so
running both concurrently slows DVE ~2.7x (measured) and nets nothing.
"""
import heapq
import numpy as np
from contextlib import ExitStack

import concourse.bass as bass
import concourse.tile as tile
from concourse import mybir
from concourse.bass_utils import run_bass_kernel_spmd

F16 = mybir.dt.float16
F32 = mybir.dt.float32
I32 = mybir.dt.int32
AF = mybir.ActivationFunctionType
OP = mybir.AluOpType
P = 128
NCORES = 8
NWC = 20                 # windows per core
NEG_SLOPE = 0.2
RS1 = 656                # layer-1 record: 256+256+8+8 + one-hot(128)
MS1 = 520                # layer-1 message block: 256 + 8(ex) + 256
RS2 = 260                # layer-2 record: 256 + 1.0 + asrc + adst + dl
SPIN = 24                # PE warm-up matmuls (HAM releases the clock gate
                         # only after ~3.4us of sustained PE activity)


def _split_excess_waits(nc, max_waits=1):
    """This walrus build rejects instructions with >~2 sync waits; move excess
    waits onto same-engine wait-only instructions placed just before."""
    cnt = 0
    for f in nc.m.functions:
        for bb in f.blocks:
            new_insts = []
            for inst in bb.instructions:
                si = inst.sync_info
                if si is not None and si.on_wait and len(si.on_wait) > max_waits:
                    waits = list(si.on_wait)
                    extra, keep = waits[:-max_waits], waits[-max_waits:]
                    for w in extra:
                        cnt += 1
                        nop = mybir.InstNoOp(name=f"wsplit-{cnt}-{inst.name}", ins=[], outs=[])
                        nop.engine = inst.engine
                        nop.sync_info = mybir.SyncInfo(on_wait=[w], on_update=[])
                        new_insts.append(nop)
                    si.on_wait = keep
                new_insts.append(inst)
            bb.instructions = new_insts
    return cnt


def _pack_windows(dst, N, nw):
    """Greedy balance in-degree over nw windows of P slots. Returns
    win_of[N], slot_of[N], K (edge chunks per window)."""
    deg = np.bincount(dst, minlength=N)
    order = np.argsort(-deg, kind="stable")
    wload = np.zeros(nw, np.int64)
    wcnt = np.zeros(nw, np.int64)
    win_of = np.zeros(N, np.int32)
    slot_of = np.zeros(N, np.int32)
    heap = [(0, w) for w in range(nw)]
    heapq.heapify(heap)
    for n in order:
        while True:
            load, w = heapq.heappop(heap)
            if wcnt[w] < P:
                break
        win_of[n] = w
        slot_of[n] = wcnt[w]
        wcnt[w] += 1
        wload[w] += deg[n]
        if wcnt[w] < P:
            heapq.heappush(heap, (wload[w], w))
    K = int(np.ceil(wload.max() / P))
    return win_of, slot_of, K


def _kgroups(K, ng):
    """Split range(K) into ng contiguous groups for DVE/PE pipelining."""
    out = []
    base = 0
    for g in range(ng):
        n = (K - base + (ng - g) - 1) // (ng - g)
        out.append((base, base + n))
        base += n
    return out


def _spin_init(nc, const):
    a = const.tile([P, P], F16, tag="spin_a")
    nc.vector.memset(a[:], 1.0)
    b = const.tile([P, 512], F16, tag="spin_b")
    nc.vector.memset(b[:], 0.5)
    return a, b


def _spin(nc, spin_ab, pool, n):
    """Dependency-free matmuls that keep the PE HAM activity monitor busy
    (the clock gate drops to 1.2 GHz after ~3.4us of low PE activity);
    issued ahead of real matmuls so they fill operand-wait gaps."""
    a, b = spin_ab
    ps = pool.tile([P, 512], F32, tag="spin_ps")
    for i in range(n):
        nc.tensor.matmul(out=ps[:], lhsT=a[:], rhs=b[:], start=i == 0,
                         stop=i == n - 1)


def _build_a(D1, NPC):
    """h = x@W1 + fused alphas. out row = [po0(264) | po1(264)] =
    [h[0:256], asrc(8), h[256:512], adst(8)]."""
    nc = bass.Bass("TRN2", target_bir_lowering=False, debug=False, num_devices=NCORES)
    xT = nc.dram_tensor("xT", [D1, NPC], F16, kind="ExternalInput")
    W1E = nc.dram_tensor("W1E", [D1, 2 * 264], F16, kind="ExternalInput")
    tab1 = nc.dram_tensor("tab1", [NPC, 528], F16, kind="ExternalOutput")
    KB = D1 // P
    with tile.TileContext(nc) as tc:
        with ExitStack() as ctx:
            const = ctx.enter_context(tc.tile_pool(name="const", bufs=1))
            work = ctx.enter_context(tc.tile_pool(name="work", bufs=3))
            ps0 = ctx.enter_context(tc.tile_pool(name="ps0", bufs=2, space="PSUM"))
            ps1 = ctx.enter_context(tc.tile_pool(name="ps1", bufs=2, space="PSUM"))
            psW = ctx.enter_context(tc.tile_pool(name="psW", bufs=1, space="PSUM"))
            spin_ab = _spin_init(nc, const)
            _spin(nc, spin_ab, psW, SPIN)
            xsb, w0sb, w1sb = [], [], []
            for kb in range(KB):
                t = const.tile([P, NPC], F16, tag=f"x_{kb}")
                nc.sync.dma_start(out=t[:], in_=xT[kb * P:(kb + 1) * P, :])
                xsb.append(t)
                t0 = const.tile([P, 264], F16, tag=f"w0_{kb}")
                nc.sync.dma_start(out=t0[:], in_=W1E[kb * P:(kb + 1) * P, 0:264])
                w0sb.append(t0)
                t1 = const.tile([P, 264], F16, tag=f"w1_{kb}")
                nc.sync.dma_start(out=t1[:], in_=W1E[kb * P:(kb + 1) * P, 264:528])
                w1sb.append(t1)
            for t_i in range(NPC // P):
                po0 = ps0.tile([P, 264], F32, tag="po0")
                po1 = ps1.tile([P, 264], F32, tag="po1")
                for kb in range(KB):
                    lhsT = xsb[kb][:, t_i * P:(t_i + 1) * P]
                    nc.tensor.matmul(out=po0[:], lhsT=lhsT, rhs=w0sb[kb][:],
                                     start=kb == 0, stop=kb == KB - 1)
                for kb in range(KB):
                    lhsT = xsb[kb][:, t_i * P:(t_i + 1) * P]
                    nc.tensor.matmul(out=po1[:], lhsT=lhsT, rhs=w1sb[kb][:],
                                     start=kb == 0, stop=kb == KB - 1)
                stage = work.tile([P, 528], F16, tag="stage")
                nc.scalar.activation(out=stage[:, 0:264], in_=po0[:], func=AF.Copy)
                nc.vector.tensor_copy(out=stage[:, 264:528], in_=po1[:])
                nc.sync.dma_start(out=tab1[t_i * P:(t_i + 1) * P, :], in_=stage[:])
    _split_excess_waits(nc)
    return nc


def _build_b(K, D1, H1, OUTC, add_bias):
    """Layer-1 edge phase + flush into the layer-2 table.

    Elementwise work split: DVE keeps msg_lo scaling + part of the selector
    compares + the (PSUM-bound) flush; GpSimd takes msg_hi scaling, the rest
    of the compares, and the small copies. Both sliced into 4 k-groups so
    selector matmuls start early and the PE never idles long enough for the
    HAM clock gate to drop."""
    C1 = D1 // H1
    nc = bass.Bass("TRN2", target_bir_lowering=False, debug=False, num_devices=NCORES)
    EDG = nc.dram_tensor("EDG", [NWC, P, K * RS1], F16, kind="ExternalInput")
    W2E = nc.dram_tensor("W2E", [D1, OUTC + 2], F16, kind="ExternalInput")
    BB = nc.dram_tensor("BB", [P, D1], F32, kind="ExternalInput")
    tab2 = nc.dram_tensor("tab2", [NWC * P, OUTC + 2], F16, kind="ExternalOutput")
    KG = _kgroups(K, 4)
    with tile.TileContext(nc) as tc:
        with ExitStack() as ctx:
            const = ctx.enter_context(tc.tile_pool(name="const", bufs=1))
            gp = ctx.enter_context(tc.tile_pool(name="gp", bufs=4))
            mp = ctx.enter_context(tc.tile_pool(name="mp", bufs=3))
            cp = ctx.enter_context(tc.tile_pool(name="cp", bufs=2))
            sp = ctx.enter_context(tc.tile_pool(name="sp", bufs=3))
            fp = ctx.enter_context(tc.tile_pool(name="fp", bufs=2))
            st = ctx.enter_context(tc.tile_pool(name="st", bufs=2))
            ps0 = ctx.enter_context(tc.tile_pool(name="ps0", bufs=2, space="PSUM"))
            ps1 = ctx.enter_context(tc.tile_pool(name="ps1", bufs=2, space="PSUM"))
            psD = ctx.enter_context(tc.tile_pool(name="psD", bufs=2, space="PSUM"))
            psH = ctx.enter_context(tc.tile_pool(name="psH", bufs=1, space="PSUM"))
            psT = ctx.enter_context(tc.tile_pool(name="psT", bufs=1, space="PSUM"))

            iota_i = const.tile([P, P], I32)
            nc.gpsimd.iota(iota_i[:], pattern=[[1, P]], base=0, channel_multiplier=0)
            piota_i = const.tile([P, 1], I32)
            nc.gpsimd.iota(piota_i[:], pattern=[[0, 1]], base=0, channel_multiplier=1)
            piota_f = const.tile([P, 1], F32)
            nc.vector.tensor_copy(out=piota_f[:], in_=piota_i[:])
            iota_f = const.tile([P, P], F32)
            nc.vector.tensor_copy(out=iota_f[:], in_=iota_i[:])
            identF = const.tile([P, P], F32)
            nc.vector.tensor_tensor(out=identF[:], in0=iota_f[:],
                                    in1=piota_f[:].to_broadcast([P, P]), op=OP.is_equal)
            if add_bias:
                bb = const.tile([P, D1], F32)
                nc.sync.dma_start(out=bb[:], in_=BB[:, :])
            w2e_sb = []
            for cb in range(D1 // P):
                t = const.tile([P, OUTC + 2], F16, tag=f"w2e_{cb}")
                nc.sync.dma_start(out=t[:], in_=W2E[cb * P:(cb + 1) * P, :])
                w2e_sb.append(t)

            def flush_b(w, po0, po1, pd):
                dr = fp.tile([P, H1], F32, tag="dr")
                nc.scalar.activation(out=dr[:], in_=pd[:], func=AF.Copy,
                                     bias=1e-16)
                drr = fp.tile([P, H1], F32, tag="drr")
                nc.vector.reciprocal(out=drr[:], in_=dr[:])
                o1 = fp.tile([P, D1], F32, tag="o1")
                nc.vector.tensor_tensor(
                    out=o1[:, 0:256].rearrange("p (h c) -> p h c", h=4),
                    in0=po0[:].rearrange("p (h c) -> p h c", h=4),
                    in1=drr[:, 0:4].to_broadcast([P, 4, C1]), op=OP.mult)
                nc.vector.tensor_tensor(
                    out=o1[:, 256:512].rearrange("p (h c) -> p h c", h=4),
                    in0=po1[:].rearrange("p (h c) -> p h c", h=4),
                    in1=drr[:, 4:8].to_broadcast([P, 4, C1]), op=OP.mult)
                if add_bias:
                    nc.vector.tensor_tensor(out=o1[:], in0=o1[:], in1=bb[:], op=OP.add)
                ee = fp.tile([P, D1], F32, tag="ee")
                nc.scalar.activation(out=ee[:], in_=o1[:], func=AF.Exp)
                nc.vector.tensor_scalar(out=ee[:], in0=ee[:], scalar1=1.0,
                                        scalar2=-1.0, op0=OP.min, op1=OP.add)
                h2 = fp.tile([P, D1], F32, tag="h2")
                nc.vector.tensor_tensor(out=h2[:], in0=o1[:], in1=ee[:], op=OP.max)
                ph2 = psH.tile([P, OUTC + 2], F32, tag="ph2")
                for cb in range(D1 // P):
                    pt = psT.tile([P, P], F32, tag="pt")
                    nc.tensor.transpose(out=pt[:], in_=h2[:, cb * P:(cb + 1) * P],
                                        identity=identF[:])
                    h2t = cp.tile([P, P], F16, tag="h2t")
                    nc.scalar.activation(out=h2t[:], in_=pt[:], func=AF.Copy)
                    nc.tensor.matmul(out=ph2[:], lhsT=h2t[:], rhs=w2e_sb[cb][:],
                                     start=cb == 0, stop=cb == D1 // P - 1)
                stage = st.tile([P, OUTC + 2], F16, tag="stage")
                nc.scalar.activation(out=stage[:], in_=ph2[:], func=AF.Copy)
                nc.sync.dma_start(out=tab2[w * P:(w + 1) * P, :], in_=stage[:])

            MB = K * 512              # meta region base: [asrc 8 | adst 8 | onehot 128]
            pending = []
            for w in range(NWC):
                G = gp.tile([P, K * RS1], F16, tag="G")
                nc.sync.dma_start(out=G[:], in_=EDG[w])
                Gm = G[:, MB:].rearrange("p (k t) -> p k t", t=144)
                S = sp.tile([P, K * H1], F32, tag="S")
                nc.vector.tensor_tensor(
                    out=S[:].rearrange("p (k h) -> p k h", h=H1),
                    in0=Gm[:, :, 0:8], in1=Gm[:, :, 8:16], op=OP.add)
                LR = sp.tile([P, K * H1], F32, tag="LR")
                nc.scalar.activation(out=LR[:], in_=S[:], func=AF.Prelu, alpha=NEG_SLOPE)
                EX = sp.tile([P, K * H1], F16, tag="EX")
                nc.scalar.activation(out=EX[:], in_=LR[:], func=AF.Exp)

                po0 = ps0.tile([P, 256], F32, tag="po0")
                po1 = ps1.tile([P, 256], F32, tag="po1")
                pd = psD.tile([P, H1], F32, tag="pd")
                for g, (k0, k1) in enumerate(KG):
                    L = k1 - k0
                    Mg = mp.tile([P, L * 512], F16, tag=f"M{g}")
                    nc.vector.tensor_tensor(
                        out=Mg[:].rearrange("p (j c) -> p j c", c=C1),
                        in0=G[:, k0 * 512:k1 * 512].rearrange("p (j c) -> p j c", c=C1),
                        in1=EX[:, k0 * H1:k1 * H1].to_broadcast([P, L * H1, C1]),
                        op=OP.mult)
                    for k in range(L):
                        gk = k0 + k
                        lhsT = G[:, MB + gk * 144 + 16:MB + (gk + 1) * 144]
                        nc.tensor.matmul(out=po0[:], lhsT=lhsT,
                                         rhs=Mg[:, k * 512:k * 512 + 256],
                                         start=gk == 0, stop=gk == K - 1)
                        nc.tensor.matmul(out=po1[:], lhsT=lhsT,
                                         rhs=Mg[:, k * 512 + 256:(k + 1) * 512],
                                         start=gk == 0, stop=gk == K - 1)
                        nc.tensor.matmul(out=pd[:], lhsT=lhsT,
                                         rhs=EX[:, gk * H1:(gk + 1) * H1],
                                         start=gk == 0, stop=gk == K - 1)

                pending.append((w, po0, po1, pd))
                if len(pending) > 1:
                    flush_b(*pending.pop(0))
            flush_b(*pending.pop(0))
    _split_excess_waits(nc)
    return nc


def _build_c(K, OUTC, add_bias):
    """Layer-2 edge phase: coefficient folded into the selector
    (CMPX = onehot * ex), raw message rows stream straight into the
    matmul; flush = divide, (+b2,) log_softmax."""
    nc = bass.Bass("TRN2", target_bir_lowering=False, debug=False, num_devices=NCORES)
    EDG = nc.dram_tensor("EDG", [NWC, P, K * RS2], F16, kind="ExternalInput")
    BB = nc.dram_tensor("BB", [P, OUTC], F32, kind="ExternalInput")
    out_t = nc.dram_tensor("out", [NWC * P, OUTC], F32, kind="ExternalOutput")
    KG = _kgroups(K, 4)
    with tile.TileContext(nc) as tc:
        with ExitStack() as ctx:
            const = ctx.enter_context(tc.tile_pool(name="const", bufs=1))
            gp = ctx.enter_context(tc.tile_pool(name="gp", bufs=4))
            cp = ctx.enter_context(tc.tile_pool(name="cp", bufs=3))
            sp = ctx.enter_context(tc.tile_pool(name="sp", bufs=3))
            fp = ctx.enter_context(tc.tile_pool(name="fp", bufs=2))
            ps0 = ctx.enter_context(tc.tile_pool(name="ps0", bufs=2, space="PSUM"))
            psW = ctx.enter_context(tc.tile_pool(name="psW", bufs=1, space="PSUM"))
            spin_ab = _spin_init(nc, const)
            _spin(nc, spin_ab, psW, SPIN)

            iota_i = const.tile([P, P], I32)
            nc.gpsimd.iota(iota_i[:], pattern=[[1, P]], base=0, channel_multiplier=0)
            iotag = const.tile([P, K * P], F16)
            for k in range(K):
                nc.gpsimd.tensor_copy(out=iotag[:, k * P:(k + 1) * P], in_=iota_i[:])
            if add_bias:
                bb = const.tile([P, OUTC], F32)
                nc.sync.dma_start(out=bb[:], in_=BB[:, :])

            def flush_c(w, po):
                dr = fp.tile([P, 1], F32, tag="dr")
                nc.scalar.activation(out=dr[:], in_=po[:, 256:257], func=AF.Copy,
                                     bias=1e-16)
                drr = fp.tile([P, 1], F32, tag="drr")
                nc.vector.reciprocal(out=drr[:], in_=dr[:])
                z = fp.tile([P, OUTC], F32, tag="z")
                nc.scalar.activation(out=z[:], in_=po[:, 0:256], func=AF.Copy,
                                     scale=drr[:, :1])
                if add_bias:
                    nc.vector.tensor_tensor(out=z[:], in0=z[:], in1=bb[:], op=OP.add)
                ee = fp.tile([P, OUTC], F32, tag="ee")
                se = fp.tile([P, 1], F32, tag="se")
                nc.scalar.activation(out=ee[:], in_=z[:], func=AF.Exp, accum_out=se[:])
                lse = fp.tile([P, 1], F32, tag="lse")
                nc.scalar.activation(out=lse[:], in_=se[:], func=AF.Ln)
                nc.vector.tensor_scalar(out=z[:], in0=z[:], scalar1=lse[:, :1],
                                        scalar2=None, op0=OP.subtract)
                nc.sync.dma_start(out=out_t[w * P:(w + 1) * P, :], in_=z[:])

            pending = []
            for w in range(NWC):
                G = gp.tile([P, K * RS2], F16, tag="G")
                nc.sync.dma_start(out=G[:], in_=EDG[w])
                Gv = G[:].rearrange("p (k t) -> p k t", t=RS2)
                S = sp.tile([P, K], F32, tag="S")
                nc.vector.tensor_tensor(
                    out=S[:].rearrange("p (k o) -> p k o", o=1),
                    in0=Gv[:, :, 257:258], in1=Gv[:, :, 258:259], op=OP.add)
                LR = sp.tile([P, K], F32, tag="LR")
                nc.scalar.activation(out=LR[:], in_=S[:], func=AF.Prelu, alpha=NEG_SLOPE)
                EX = sp.tile([P, K], F16, tag="EX")
                nc.scalar.activation(out=EX[:], in_=LR[:], func=AF.Exp)
                EXv = EX[:].rearrange("p (k o) -> p k o", o=1)

                po = ps0.tile([P, 257], F32, tag="po")
                for g, (k0, k1) in enumerate(KG):
                    L = k1 - k0
                    _spin(nc, spin_ab, psW, 2)
                    CMPg = cp.tile([P, L * P], F16, tag=f"C{g}")
                    nc.vector.tensor_tensor(
                        out=CMPg[:].rearrange("p (k q) -> p k q", q=P),
                        in0=iotag[:, k0 * P:k1 * P].rearrange("p (k q) -> p k q", q=P),
                        in1=Gv[:, k0:k1, 259:260].rearrange("p k o -> p (k o)")
                            .to_broadcast([P, L, P]),
                        op=OP.is_equal)
                    CMXg = cp.tile([P, L * P], F16, tag=f"X{g}")
                    nc.vector.tensor_tensor(
                        out=CMXg[:].rearrange("p (k q) -> p k q", q=P),
                        in0=CMPg[:].rearrange("p (k q) -> p k q", q=P),
                        in1=EXv[:, k0:k1, :].to_broadcast([P, L, P]), op=OP.mult)
                    for k in range(L):
                        gk = k0 + k
                        nc.tensor.matmul(out=po[:], lhsT=CMXg[:, k * P:(k + 1) * P],
                                         rhs=G[:, gk * RS2:gk * RS2 + 257],
                                         start=gk == 0, stop=gk == K - 1)

                pending.append((w, po))
                if len(pending) > 1:
                    flush_c(*pending.pop(0))
            flush_c(*pending.pop(0))
    _split_excess_waits(nc)
    return nc


def kernel(x, edge_index, W1, att_src1, att_dst1, b1, W2, att_src2, att_dst2, b2):
    x = np.asarray(x, np.float32)
    edge_index = np.asarray(edge_index)
    W1d = np.asarray(W1, np.float64)
    W2d = np.asarray(W2, np.float64)
    as1 = np.asarray(att_src1, np.float64)
    ad1 = np.asarray(att_dst1, np.float64)
    as2 = np.asarray(att_src2, np.float64)
    ad2 = np.asarray(att_dst2, np.float64)
    b1 = np.asarray(b1, np.float32)
    b2 = np.asarray(b2, np.float32)
    N, D1 = x.shape
    H1, C1 = att_src1.shape
    OUTC = W2.shape[1]
    NW = NCORES * NWC
    NPC = NWC * P
    core_ids = list(range(NCORES))
    npc_in = N // NCORES

    src = np.concatenate([edge_index[0], np.arange(N)]).astype(np.int64)
    dst = np.concatenate([edge_index[1], np.arange(N)]).astype(np.int64)
    win_of, slot_of, K = _pack_windows(dst, N, NW)

    # edge -> (window, chunk, partition) in window-major stable order
    w_e = win_of[dst]
    eorder = np.argsort(w_e, kind="stable")
    sw = w_e[eorder]
    counts = np.bincount(sw, minlength=NW)
    starts = np.concatenate([[0], np.cumsum(counts)[:-1]])
    pos = np.arange(len(sw)) - starts[sw]
    k_e = (pos // P).astype(np.int64)
    p_e = (pos % P).astype(np.int64)
    s_e = src[eorder]
    d_e = dst[eorder]
    row_of_node = win_of.astype(np.int64) * P + slot_of  # global table row

    # ---- Launch A: per-node table [h_lo, asrc, h_hi, adst] ----
    asd_s = np.zeros((D1, H1))
    asd_d = np.zeros((D1, H1))
    for h in range(H1):
        asd_s[h * C1:(h + 1) * C1, h] = as1[h]
        asd_d[h * C1:(h + 1) * C1, h] = ad1[h]
    W1E = np.concatenate([W1d[:, 0:256], W1d @ asd_s, W1d[:, 256:512], W1d @ asd_d],
                         axis=1).astype(np.float16)
    nc_a = _build_a(D1, NPC)
    in_maps = []
    for c in range(NCORES):
        xo = np.zeros((NPC, D1), np.float16)
        xo[:npc_in] = x[c * npc_in:(c + 1) * npc_in].astype(np.float16)
        in_maps.append({"xT": np.ascontiguousarray(xo.T), "W1E": W1E})
    res_a = run_bass_kernel_spmd(nc_a, in_maps, core_ids)
    tab1 = np.concatenate([res_a.results[c]["tab1"][:npc_in] for c in range(NCORES)], axis=0)
    h_lo = tab1[:, 0:256]
    a_src_n = tab1[:, 256:264]
    h_hi = tab1[:, 264:520]
    a_dst_n = tab1[:, 520:528]

    # ---- Launch B: layer-1 edge phase ----
    W2E = np.concatenate([W2d, W2d @ as2.T, W2d @ ad2.T], axis=1).astype(np.float16)
    BB1 = np.tile(b1.reshape(1, D1), (P, 1))
    nc_b = _build_b(K, D1, H1, OUTC, bool(np.any(b1)))
    in_maps = []
    for c in range(NCORES):
        m = (sw >= c * NWC) & (sw < (c + 1) * NWC)
        lw, kk, pp = sw[m] - c * NWC, k_e[m], p_e[m]
        sm, dm = s_e[m], d_e[m]
        EDGm = np.zeros((NWC, P, K, 512), np.float16)
        EDGr = np.zeros((NWC, P, K, 144), np.float16)
        EDGm[lw, pp, kk, 0:256] = h_lo[sm]
        EDGm[lw, pp, kk, 256:512] = h_hi[sm]
        EDGr[lw, pp, kk, 0:8] = a_src_n[sm]
        EDGr[lw, pp, kk, 8:16] = a_dst_n[dm]
        EDGr[lw, pp, kk, 16 + slot_of[dm]] = 1.0
        EDG = np.ascontiguousarray(np.concatenate(
            [EDGm.reshape(NWC, P, K * 512), EDGr.reshape(NWC, P, K * 144)], axis=2))
        in_maps.append({"EDG": EDG, "W2E": W2E, "BB": BB1})
    res_b = run_bass_kernel_spmd(nc_b, in_maps, core_ids)
    tab2 = np.concatenate([res_b.results[c]["tab2"] for c in range(NCORES)], axis=0)
    h2p = tab2[:, 0:256]
    a_src2_n = tab2[:, 256]
    a_dst2_n = tab2[:, 257]

    # ---- Launch C: layer-2 edge phase + log_softmax ----
    BB2 = np.tile(b2.reshape(1, OUTC), (P, 1))
    nc_c = _build_c(K, OUTC, bool(np.any(b2)))
    in_maps = []
    sr = row_of_node[s_e]
    dr_ = row_of_node[d_e]
    for c in range(NCORES):
        m = (sw >= c * NWC) & (sw < (c + 1) * NWC)
        lw, kk, pp = sw[m] - c * NWC, k_e[m], p_e[m]
        srm, drm = sr[m], dr_[m]
        EDG = np.zeros((NWC, P, K, RS2), np.float16)
        EDG[:, :, :, 259] = 255.0
        EDG[lw, pp, kk, 0:256] = h2p[srm]
        EDG[lw, pp, kk, 256] = 1.0
        EDG[lw, pp, kk, 257] = a_src2_n[srm]
        EDG[lw, pp, kk, 258] = a_dst2_n[drm]
        EDG[lw, pp, kk, 259] = slot_of[d_e[m]].astype(np.float16)
        in_maps.append({"EDG": EDG.reshape(NWC, P, K * RS2), "BB": BB2})
    res_c = run_bass_kernel_spmd(nc_c, in_maps, core_ids)
    rows = np.concatenate([res_c.results[c]["out"] for c in range(NCORES)], axis=0)
    return np.ascontiguousarray(rows[row_of_node]).astype(np.float32)


# revision 23
# speedup vs baseline: 1.0591x; 1.0056x over previous
"""2-layer GAT (GATConv x2 + log_softmax) on 8 Trainium2 NeuronCores.

fp16 streaming design (SPMD across 8 cores; host does index marshaling
between launches, device does all arithmetic):
  - Host bin-packs the 20000 nodes into 160 windows of 128 dst slots each
    (20 windows/core), balancing in-degree so every window holds ~2125
    edges -> K = ceil(max/128) = 17 chunks of 128 edges.
  - Launch A: h = x@W1 with the attention alphas fused in via host-extended
    weights [W1 | W1@asd_src | W1@asd_dst]; emits a per-node fp16 table.
  - Host pre-gathers per-edge rows (halo exchange): for each edge slot a
    656-col fp16 record [h_lo(256) | h_hi(256) | asrc(8) | adst(8) |
    one-hot dst selector(128)].
  - Launch B: per window one bulk DMA; ACT computes exp(leakyrelu(alpha)),
    DVE scales messages by the per-head coefficients (sliced per k-group so
    selector matmuls start early); PE scatter-adds messages+denominators
    into PSUM via the shipped selectors; flush: divide, ELU, @[W2 | W2@a2]
    -> layer-2 fp16 table rows.
  - Launch C: layer 2 (H=1) folds the coefficient into the selector
    (CMPX = onehot(dl) * ex), so the 256-wide messages stream raw into the
    matmul; flush = log_softmax.
All matmul operands fp16 (1 cyc/row, f32 PSUM accumulation). All big
elementwise work stays on DVE: GpSimd shares SBUF ports with it, so
running both concurrently slows DVE ~2.7x (measured) and nets nothing.
"""
import heapq
import numpy as np
from contextlib import ExitStack

import concourse.bass as bass
import concourse.tile as tile
from concourse import mybir
from concourse.bass_utils import run_bass_kernel_spmd

F16 = mybir.dt.float16
F32 = mybir.dt.float32
I32 = mybir.dt.int32
AF = mybir.ActivationFunctionType
OP = mybir.AluOpType
P = 128
NCORES = 8
NWC = 20                 # windows per core
NEG_SLOPE = 0.2
RS1 = 656                # layer-1 record: 256+256+8+8 + one-hot(128)
MS1 = 520                # layer-1 message block: 256 + 8(ex) + 256
RS2 = 260                # layer-2 record: 256 + 1.0 + asrc + adst + dl
SPIN = 24                # PE warm-up matmuls (HAM releases the clock gate
                         # only after ~3.4us of sustained PE activity)


def _split_excess_waits(nc, max_waits=1):
    """This walrus build rejects instructions with >~2 sync waits; move excess
    waits onto same-engine wait-only instructions placed just before."""
    cnt = 0
    for f in nc.m.functions:
        for bb in f.blocks:
            new_insts = []
            for inst in bb.instructions:
                si = inst.sync_info
                if si is not None and si.on_wait and len(si.on_wait) > max_waits:
                    waits = list(si.on_wait)
                    extra, keep = waits[:-max_waits], waits[-max_waits:]
                    for w in extra:
                        cnt += 1
                        nop = mybir.InstNoOp(name=f"wsplit-{cnt}-{inst.name}", ins=[], outs=[])
                        nop.engine = inst.engine
                        nop.sync_info = mybir.SyncInfo(on_wait=[w], on_update=[])
                        new_insts.append(nop)
                    si.on_wait = keep
                new_insts.append(inst)
            bb.instructions = new_insts
    return cnt


def _pack_windows(dst, N, nw):
    """Greedy balance in-degree over nw windows of P slots. Returns
    win_of[N], slot_of[N], K (edge chunks per window)."""
    deg = np.bincount(dst, minlength=N)
    order = np.argsort(-deg, kind="stable")
    wload = np.zeros(nw, np.int64)
    wcnt = np.zeros(nw, np.int64)
    win_of = np.zeros(N, np.int32)
    slot_of = np.zeros(N, np.int32)
    heap = [(0, w) for w in range(nw)]
    heapq.heapify(heap)
    for n in order:
        while True:
            load, w = heapq.heappop(heap)
            if wcnt[w] < P:
                break
        win_of[n] = w
        slot_of[n] = wcnt[w]
        wcnt[w] += 1
        wload[w] += deg[n]
        if wcnt[w] < P:
            heapq.heappush(heap, (wload[w], w))
    K = int(np.ceil(wload.max() / P))
    return win_of, slot_of, K


def _kgroups(K, ng):
    """Split range(K) into ng contiguous groups for DVE/PE pipelining."""
    out = []
    base = 0
    for g in range(ng):
        n = (K - base + (ng - g) - 1) // (ng - g)
        out.append((base, base + n))
        base += n
    return out


def _spin_init(nc, const):
    a = const.tile([P, P], F16, tag="spin_a")
    nc.vector.memset(a[:], 1.0)
    b = const.tile([P, 512], F16, tag="spin_b")
    nc.vector.memset(b[:], 0.5)
    return a, b


def _spin(nc, spin_ab, pool, n):
    """Dependency-free matmuls that keep the PE HAM activity monitor busy
    (the clock gate drops to 1.2 GHz after ~3.4us of low PE activity);
    issued ahead of real matmuls so they fill operand-wait gaps."""
    a, b = spin_ab
    ps = pool.tile([P, 512], F32, tag="spin_ps")
    for i in range(n):
        nc.tensor.matmul(out=ps[:], lhsT=a[:], rhs=b[:], start=i == 0,
                         stop=i == n - 1)


def _build_a(D1, NPC):
    """h = x@W1 + fused alphas. out row = [po0(264) | po1(264)] =
    [h[0:256], asrc(8), h[256:512], adst(8)]."""
    nc = bass.Bass("TRN2", target_bir_lowering=False, debug=False, num_devices=NCORES)
    xT = nc.dram_tensor("xT", [D1, NPC], F16, kind="ExternalInput")
    W1E = nc.dram_tensor("W1E", [D1, 2 * 264], F16, kind="ExternalInput")
    tab1 = nc.dram_tensor("tab1", [NPC, 528], F16, kind="ExternalOutput")
    KB = D1 // P
    with tile.TileContext(nc) as tc:
        with ExitStack() as ctx:
            const = ctx.enter_context(tc.tile_pool(name="const", bufs=1))
            work = ctx.enter_context(tc.tile_pool(name="work", bufs=3))
            ps0 = ctx.enter_context(tc.tile_pool(name="ps0", bufs=2, space="PSUM"))
            ps1 = ctx.enter_context(tc.tile_pool(name="ps1", bufs=2, space="PSUM"))
            psW = ctx.enter_context(tc.tile_pool(name="psW", bufs=1, space="PSUM"))
            spin_ab = _spin_init(nc, const)
            _spin(nc, spin_ab, psW, SPIN)
            xsb, w0sb, w1sb = [], [], []
            for kb in range(KB):
                t = const.tile([P, NPC], F16, tag=f"x_{kb}")
                nc.sync.dma_start(out=t[:], in_=xT[kb * P:(kb + 1) * P, :])
                xsb.append(t)
                t0 = const.tile([P, 264], F16, tag=f"w0_{kb}")
                nc.sync.dma_start(out=t0[:], in_=W1E[kb * P:(kb + 1) * P, 0:264])
                w0sb.append(t0)
                t1 = const.tile([P, 264], F16, tag=f"w1_{kb}")
                nc.sync.dma_start(out=t1[:], in_=W1E[kb * P:(kb + 1) * P, 264:528])
                w1sb.append(t1)
            for t_i in range(NPC // P):
                po0 = ps0.tile([P, 264], F32, tag="po0")
                po1 = ps1.tile([P, 264], F32, tag="po1")
                for kb in range(KB):
                    lhsT = xsb[kb][:, t_i * P:(t_i + 1) * P]
                    nc.tensor.matmul(out=po0[:], lhsT=lhsT, rhs=w0sb[kb][:],
                                     start=kb == 0, stop=kb == KB - 1)
                for kb in range(KB):
                    lhsT = xsb[kb][:, t_i * P:(t_i + 1) * P]
                    nc.tensor.matmul(out=po1[:], lhsT=lhsT, rhs=w1sb[kb][:],
                                     start=kb == 0, stop=kb == KB - 1)
                stage = work.tile([P, 528], F16, tag="stage")
                nc.scalar.activation(out=stage[:, 0:264], in_=po0[:], func=AF.Copy)
                nc.vector.tensor_copy(out=stage[:, 264:528], in_=po1[:])
                nc.sync.dma_start(out=tab1[t_i * P:(t_i + 1) * P, :], in_=stage[:])
    _split_excess_waits(nc)
    return nc


def _build_b(K, D1, H1, OUTC, add_bias):
    """Layer-1 edge phase + flush into the layer-2 table.

    Elementwise work split: DVE keeps msg_lo scaling + part of the selector
    compares + the (PSUM-bound) flush; GpSimd takes msg_hi scaling, the rest
    of the compares, and the small copies. Both sliced into 4 k-groups so
    selector matmuls start early and the PE never idles long enough for the
    HAM clock gate to drop."""
    C1 = D1 // H1
    nc = bass.Bass("TRN2", target_bir_lowering=False, debug=False, num_devices=NCORES)
    EDG = nc.dram_tensor("EDG", [NWC, P, K * RS1], F16, kind="ExternalInput")
    W2E = nc.dram_tensor("W2E", [D1, OUTC + 2], F16, kind="ExternalInput")
    BB = nc.dram_tensor("BB", [P, D1], F32, kind="ExternalInput")
    tab2 = nc.dram_tensor("tab2", [NWC * P, OUTC + 2], F16, kind="ExternalOutput")
    KG = _kgroups(K, 4)
    with tile.TileContext(nc) as tc:
        with ExitStack() as ctx:
            const = ctx.enter_context(tc.tile_pool(name="const", bufs=1))
            gp = ctx.enter_context(tc.tile_pool(name="gp", bufs=4))
            mp = ctx.enter_context(tc.tile_pool(name="mp", bufs=3))
            cp = ctx.enter_context(tc.tile_pool(name="cp", bufs=2))
            sp = ctx.enter_context(tc.tile_pool(name="sp", bufs=3))
            fp = ctx.enter_context(tc.tile_pool(name="fp", bufs=2))
            st = ctx.enter_context(tc.tile_pool(name="st", bufs=2))
            ps0 = ctx.enter_context(tc.tile_pool(name="ps0", bufs=2, space="PSUM"))
            ps1 = ctx.enter_context(tc.tile_pool(name="ps1", bufs=2, space="PSUM"))
            psD = ctx.enter_context(tc.tile_pool(name="psD", bufs=2, space="PSUM"))
            psH = ctx.enter_context(tc.tile_pool(name="psH", bufs=1, space="PSUM"))
            psT = ctx.enter_context(tc.tile_pool(name="psT", bufs=1, space="PSUM"))

            iota_i = const.tile([P, P], I32)
            nc.gpsimd.iota(iota_i[:], pattern=[[1, P]], base=0, channel_multiplier=0)
            piota_i = const.tile([P, 1], I32)
            nc.gpsimd.iota(piota_i[:], pattern=[[0, 1]], base=0, channel_multiplier=1)
            piota_f = const.tile([P, 1], F32)
            nc.vector.tensor_copy(out=piota_f[:], in_=piota_i[:])
            iota_f = const.tile([P, P], F32)
            nc.vector.tensor_copy(out=iota_f[:], in_=iota_i[:])
            identF = const.tile([P, P], F32)
            nc.vector.tensor_tensor(out=identF[:], in0=iota_f[:],
                                    in1=piota_f[:].to_broadcast([P, P]), op=OP.is_equal)
            if add_bias:
                bb = const.tile([P, D1], F32)
                nc.sync.dma_start(out=bb[:], in_=BB[:, :])
            w2e_sb = []
            for cb in range(D1 // P):
                t = const.tile([P, OUTC + 2], F16, tag=f"w2e_{cb}")
                nc.sync.dma_start(out=t[:], in_=W2E[cb * P:(cb + 1) * P, :])
                w2e_sb.append(t)

            def flush_b(w, po0, po1, pd):
                dr = fp.tile([P, H1], F32, tag="dr")
                nc.scalar.activation(out=dr[:], in_=pd[:], func=AF.Copy,
                                     bias=1e-16)
                drr = fp.tile([P, H1], F32, tag="drr")
                nc.vector.reciprocal(out=drr[:], in_=dr[:])
                o1 = fp.tile([P, D1], F32, tag="o1")
                nc.vector.tensor_tensor(
                    out=o1[:, 0:256].rearrange("p (h c) -> p h c", h=4),
                    in0=po0[:].rearrange("p (h c) -> p h c", h=4),
                    in1=drr[:, 0:4].to_broadcast([P, 4, C1]), op=OP.mult)
                nc.vector.tensor_tensor(
                    out=o1[:, 256:512].rearrange("p (h c) -> p h c", h=4),
                    in0=po1[:].rearrange("p (h c) -> p h c", h=4),
                    in1=drr[:, 4:8].to_broadcast([P, 4, C1]), op=OP.mult)
                if add_bias:
                    nc.vector.tensor_tensor(out=o1[:], in0=o1[:], in1=bb[:], op=OP.add)
                ee = fp.tile([P, D1], F32, tag="ee")
                nc.scalar.activation(out=ee[:], in_=o1[:], func=AF.Exp)
                nc.vector.tensor_scalar(out=ee[:], in0=ee[:], scalar1=1.0,
                                        scalar2=-1.0, op0=OP.min, op1=OP.add)
                h2 = fp.tile([P, D1], F32, tag="h2")
                nc.vector.tensor_tensor(out=h2[:], in0=o1[:], in1=ee[:], op=OP.max)
                ph2 = psH.tile([P, OUTC + 2], F32, tag="ph2")
                for cb in range(D1 // P):
                    pt = psT.tile([P, P], F32, tag="pt")
                    nc.tensor.transpose(out=pt[:], in_=h2[:, cb * P:(cb + 1) * P],
                                        identity=identF[:])
                    h2t = cp.tile([P, P], F16, tag="h2t")
                    nc.scalar.activation(out=h2t[:], in_=pt[:], func=AF.Copy)
                    nc.tensor.matmul(out=ph2[:], lhsT=h2t[:], rhs=w2e_sb[cb][:],
                                     start=cb == 0, stop=cb == D1 // P - 1)
                stage = st.tile([P, OUTC + 2], F16, tag="stage")
                nc.scalar.activation(out=stage[:], in_=ph2[:], func=AF.Copy)
                nc.sync.dma_start(out=tab2[w * P:(w + 1) * P, :], in_=stage[:])

            MB = K * 512              # meta region base: [asrc 8 | adst 8 | onehot 128]
            pending = []
            for w in range(NWC):
                G = gp.tile([P, K * RS1], F16, tag="G")
                nc.sync.dma_start(out=G[:], in_=EDG[w])
                Gm = G[:, MB:].rearrange("p (k t) -> p k t", t=144)
                S = sp.tile([P, K * H1], F32, tag="S")
                nc.vector.tensor_tensor(
                    out=S[:].rearrange("p (k h) -> p k h", h=H1),
                    in0=Gm[:, :, 0:8], in1=Gm[:, :, 8:16], op=OP.add)
                LR = sp.tile([P, K * H1], F32, tag="LR")
                nc.scalar.activation(out=LR[:], in_=S[:], func=AF.Prelu, alpha=NEG_SLOPE)
                EX = sp.tile([P, K * H1], F16, tag="EX")
                nc.scalar.activation(out=EX[:], in_=LR[:], func=AF.Exp)

                po0 = ps0.tile([P, 256], F32, tag="po0")
                po1 = ps1.tile([P, 256], F32, tag="po1")
                pd = psD.tile([P, H1], F32, tag="pd")
                for g, (k0, k1) in enumerate(KG):
                    L = k1 - k0
                    Mg = mp.tile([P, L * 512], F16, tag=f"M{g}")
                    nc.vector.tensor_tensor(
                        out=Mg[:].rearrange("p (j c) -> p j c", c=C1),
                        in0=G[:, k0 * 512:k1 * 512].rearrange("p (j c) -> p j c", c=C1),
                        in1=EX[:, k0 * H1:k1 * H1].to_broadcast([P, L * H1, C1]),
                        op=OP.mult)
                    for k in range(L):
                        gk = k0 + k
                        lhsT = G[:, MB + gk * 144 + 16:MB + (gk + 1) * 144]
                        nc.tensor.matmul(out=po0[:], lhsT=lhsT,
                                         rhs=Mg[:, k * 512:k * 512 + 256],
                                         start=gk == 0, stop=gk == K - 1)
                        nc.tensor.matmul(out=po1[:], lhsT=lhsT,
                                         rhs=Mg[:, k * 512 + 256:(k + 1) * 512],
                                         start=gk == 0, stop=gk == K - 1)
                        nc.tensor.matmul(out=pd[:], lhsT=lhsT,
                                         rhs=EX[:, gk * H1:(gk + 1) * H1],
                                         start=gk == 0, stop=gk == K - 1)

                pending.append((w, po0, po1, pd))
                if len(pending) > 1:
                    flush_b(*pending.pop(0))
            flush_b(*pending.pop(0))
    _split_excess_waits(nc)
    return nc


def _build_c(K, OUTC, add_bias):
    """Layer-2 edge phase: coefficient folded into the selector
    (CMPX = onehot * ex), raw message rows stream straight into the
    matmul; flush = divide, (+b2,) log_softmax."""
    nc = bass.Bass("TRN2", target_bir_lowering=False, debug=False, num_devices=NCORES)
    EDG = nc.dram_tensor("EDG", [NWC, P, K * RS2], F16, kind="ExternalInput")
    BB = nc.dram_tensor("BB", [P, OUTC], F32, kind="ExternalInput")
    out_t = nc.dram_tensor("out", [NWC * P, OUTC], F32, kind="ExternalOutput")
    KG = _kgroups(K, 4)
    with tile.TileContext(nc) as tc:
        with ExitStack() as ctx:
            const = ctx.enter_context(tc.tile_pool(name="const", bufs=1))
            gp = ctx.enter_context(tc.tile_pool(name="gp", bufs=4))
            cp = ctx.enter_context(tc.tile_pool(name="cp", bufs=3))
            sp = ctx.enter_context(tc.tile_pool(name="sp", bufs=3))
            fp = ctx.enter_context(tc.tile_pool(name="fp", bufs=2))
            ps0 = ctx.enter_context(tc.tile_pool(name="ps0", bufs=2, space="PSUM"))
            psW = ctx.enter_context(tc.tile_pool(name="psW", bufs=1, space="PSUM"))
            spin_ab = _spin_init(nc, const)
            _spin(nc, spin_ab, psW, SPIN)

            iota_i = const.tile([P, P], I32)
            nc.gpsimd.iota(iota_i[:], pattern=[[1, P]], base=0, channel_multiplier=0)
            iotag = const.tile([P, K * P], F16)
            for k in range(K):
                nc.gpsimd.tensor_copy(out=iotag[:, k * P:(k + 1) * P], in_=iota_i[:])
            if add_bias:
                bb = const.tile([P, OUTC], F32)
                nc.sync.dma_start(out=bb[:], in_=BB[:, :])

            def flush_c(w, po):
                dr = fp.tile([P, 1], F32, tag="dr")
                nc.scalar.activation(out=dr[:], in_=po[:, 256:257], func=AF.Copy,
                                     bias=1e-16)
                drr = fp.tile([P, 1], F32, tag="drr")
                nc.vector.reciprocal(out=drr[:], in_=dr[:])
                z = fp.tile([P, OUTC], F32, tag="z")
                nc.scalar.activation(out=z[:], in_=po[:, 0:256], func=AF.Copy,
                                     scale=drr[:, :1])
                if add_bias:
                    nc.vector.tensor_tensor(out=z[:], in0=z[:], in1=bb[:], op=OP.add)
                ee = fp.tile([P, OUTC], F32, tag="ee")
                se = fp.tile([P, 1], F32, tag="se")
                nc.scalar.activation(out=ee[:], in_=z[:], func=AF.Exp, accum_out=se[:])
                ser = fp.tile([P, 1], F32, tag="ser")
                nc.vector.reciprocal(out=ser[:], in_=se[:])
                nlse = fp.tile([P, 1], F32, tag="nlse")
                nc.scalar.activation(out=nlse[:], in_=ser[:], func=AF.Ln)
                z2 = fp.tile([P, OUTC], F32, tag="z2")
                nc.scalar.activation(out=z2[:], in_=z[:], func=AF.Identity,
                                     bias=nlse[:, :1])
                nc.sync.dma_start(out=out_t[w * P:(w + 1) * P, :], in_=z2[:])

            pending = []
            for w in range(NWC):
                G = gp.tile([P, K * RS2], F16, tag="G")
                nc.sync.dma_start(out=G[:], in_=EDG[w])
                Gv = G[:].rearrange("p (k t) -> p k t", t=RS2)
                S = sp.tile([P, K], F32, tag="S")
                nc.vector.tensor_tensor(
                    out=S[:].rearrange("p (k o) -> p k o", o=1),
                    in0=Gv[:, :, 257:258], in1=Gv[:, :, 258:259], op=OP.add)
                LR = sp.tile([P, K], F32, tag="LR")
                nc.scalar.activation(out=LR[:], in_=S[:], func=AF.Prelu, alpha=NEG_SLOPE)
                EX = sp.tile([P, K], F16, tag="EX")
                nc.scalar.activation(out=EX[:], in_=LR[:], func=AF.Exp)
                EXv = EX[:].rearrange("p (k o) -> p k o", o=1)

                po = ps0.tile([P, 257], F32, tag="po")
                for g, (k0, k1) in enumerate(KG):
                    L = k1 - k0
                    _spin(nc, spin_ab, psW, 2)
                    CMPg = cp.tile([P, L * P], F16, tag=f"C{g}")
                    nc.vector.tensor_tensor(
                        out=CMPg[:].rearrange("p (k q) -> p k q", q=P),
                        in0=iotag[:, k0 * P:k1 * P].rearrange("p (k q) -> p k q", q=P),
                        in1=Gv[:, k0:k1, 259:260].rearrange("p k o -> p (k o)")
                            .to_broadcast([P, L, P]),
                        op=OP.is_equal)
                    CMXg = cp.tile([P, L * P], F16, tag=f"X{g}")
                    nc.vector.tensor_tensor(
                        out=CMXg[:].rearrange("p (k q) -> p k q", q=P),
                        in0=CMPg[:].rearrange("p (k q) -> p k q", q=P),
                        in1=EXv[:, k0:k1, :].to_broadcast([P, L, P]), op=OP.mult)
                    for k in range(L):
                        gk = k0 + k
                        nc.tensor.matmul(out=po[:], lhsT=CMXg[:, k * P:(k + 1) * P],
                                         rhs=G[:, gk * RS2:gk * RS2 + 257],
                                         start=gk == 0, stop=gk == K - 1)

                pending.append((w, po))
                if len(pending) > 1:
                    flush_c(*pending.pop(0))
            flush_c(*pending.pop(0))
    _split_excess_waits(nc)
    return nc


def kernel(x, edge_index, W1, att_src1, att_dst1, b1, W2, att_src2, att_dst2, b2):
    x = np.asarray(x, np.float32)
    edge_index = np.asarray(edge_index)
    W1d = np.asarray(W1, np.float64)
    W2d = np.asarray(W2, np.float64)
    as1 = np.asarray(att_src1, np.float64)
    ad1 = np.asarray(att_dst1, np.float64)
    as2 = np.asarray(att_src2, np.float64)
    ad2 = np.asarray(att_dst2, np.float64)
    b1 = np.asarray(b1, np.float32)
    b2 = np.asarray(b2, np.float32)
    N, D1 = x.shape
    H1, C1 = att_src1.shape
    OUTC = W2.shape[1]
    NW = NCORES * NWC
    NPC = NWC * P
    core_ids = list(range(NCORES))
    npc_in = N // NCORES

    src = np.concatenate([edge_index[0], np.arange(N)]).astype(np.int64)
    dst = np.concatenate([edge_index[1], np.arange(N)]).astype(np.int64)
    win_of, slot_of, K = _pack_windows(dst, N, NW)

    # edge -> (window, chunk, partition) in window-major stable order
    w_e = win_of[dst]
    eorder = np.argsort(w_e, kind="stable")
    sw = w_e[eorder]
    counts = np.bincount(sw, minlength=NW)
    starts = np.concatenate([[0], np.cumsum(counts)[:-1]])
    pos = np.arange(len(sw)) - starts[sw]
    k_e = (pos // P).astype(np.int64)
    p_e = (pos % P).astype(np.int64)
    s_e = src[eorder]
    d_e = dst[eorder]
    row_of_node = win_of.astype(np.int64) * P + slot_of  # global table row

    # ---- Launch A: per-node table [h_lo, asrc, h_hi, adst] ----
    asd_s = np.zeros((D1, H1))
    asd_d = np.zeros((D1, H1))
    for h in range(H1):
        asd_s[h * C1:(h + 1) * C1, h] = as1[h]
        asd_d[h * C1:(h + 1) * C1, h] = ad1[h]
    W1E = np.concatenate([W1d[:, 0:256], W1d @ asd_s, W1d[:, 256:512], W1d @ asd_d],
                         axis=1).astype(np.float16)
    nc_a = _build_a(D1, NPC)
    in_maps = []
    for c in range(NCORES):
        xo = np.zeros((NPC, D1), np.float16)
        xo[:npc_in] = x[c * npc_in:(c + 1) * npc_in].astype(np.float16)
        in_maps.append({"xT": np.ascontiguousarray(xo.T), "W1E": W1E})
    res_a = run_bass_kernel_spmd(nc_a, in_maps, core_ids)
    tab1 = np.concatenate([res_a.results[c]["tab1"][:npc_in] for c in range(NCORES)], axis=0)
    h_lo = tab1[:, 0:256]
    a_src_n = tab1[:, 256:264]
    h_hi = tab1[:, 264:520]
    a_dst_n = tab1[:, 520:528]

    # ---- Launch B: layer-1 edge phase ----
    W2E = np.concatenate([W2d, W2d @ as2.T, W2d @ ad2.T], axis=1).astype(np.float16)
    BB1 = np.tile(b1.reshape(1, D1), (P, 1))
    nc_b = _build_b(K, D1, H1, OUTC, bool(np.any(b1)))
    in_maps = []
    for c in range(NCORES):
        m = (sw >= c * NWC) & (sw < (c + 1) * NWC)
        lw, kk, pp = sw[m] - c * NWC, k_e[m], p_e[m]
        sm, dm = s_e[m], d_e[m]
        EDGm = np.zeros((NWC, P, K, 512), np.float16)
        EDGr = np.zeros((NWC, P, K, 144), np.float16)
        EDGm[lw, pp, kk, 0:256] = h_lo[sm]
        EDGm[lw, pp, kk, 256:512] = h_hi[sm]
        EDGr[lw, pp, kk, 0:8] = a_src_n[sm]
        EDGr[lw, pp, kk, 8:16] = a_dst_n[dm]
        EDGr[lw, pp, kk, 16 + slot_of[dm]] = 1.0
        EDG = np.ascontiguousarray(np.concatenate(
            [EDGm.reshape(NWC, P, K * 512), EDGr.reshape(NWC, P, K * 144)], axis=2))
        in_maps.append({"EDG": EDG, "W2E": W2E, "BB": BB1})
    res_b = run_bass_kernel_spmd(nc_b, in_maps, core_ids)
    tab2 = np.concatenate([res_b.results[c]["tab2"] for c in range(NCORES)], axis=0)
    h2p = tab2[:, 0:256]
    a_src2_n = tab2[:, 256]
    a_dst2_n = tab2[:, 257]

    # ---- Launch C: layer-2 edge phase + log_softmax ----
    BB2 = np.tile(b2.reshape(1, OUTC), (P, 1))
    nc_c = _build_c(K, OUTC, bool(np.any(b2)))
    in_maps = []
    sr = row_of_node[s_e]
    dr_ = row_of_node[d_e]
    for c in range(NCORES):
        m = (sw >= c * NWC) & (sw < (c + 1) * NWC)
        lw, kk, pp = sw[m] - c * NWC, k_e[m], p_e[m]
        srm, drm = sr[m], dr_[m]
        EDG = np.zeros((NWC, P, K, RS2), np.float16)
        EDG[:, :, :, 259] = 255.0
        EDG[lw, pp, kk, 0:256] = h2p[srm]
        EDG[lw, pp, kk, 256] = 1.0
        EDG[lw, pp, kk, 257] = a_src2_n[srm]
        EDG[lw, pp, kk, 258] = a_dst2_n[drm]
        EDG[lw, pp, kk, 259] = slot_of[d_e[m]].astype(np.float16)
        in_maps.append({"EDG": EDG.reshape(NWC, P, K * RS2), "BB": BB2})
    res_c = run_bass_kernel_spmd(nc_c, in_maps, core_ids)
    rows = np.concatenate([res_c.results[c]["out"] for c in range(NCORES)], axis=0)
    return np.ascontiguousarray(rows[row_of_node]).astype(np.float32)
